# revision 53
# baseline (speedup 1.0000x reference)
"""MLA (DeepSeek-style) attention block on 8 Trainium2 NeuronCores.

Sharding:
  phase 1 (token-parallel, 8 x 512 tokens): LoRA-A down-projections + rmsnorm
    + k_pe rope; small AllGather of the kv latents (576 dims); q up-projection
    for ALL 16 heads on the token side + rope, shipped to head shards via two
    shard-aligned AllToAlls (pe+even-nope first, odd-nope second).
  phase 2 (head-parallel, 2 heads x 2 batches per core): k/v up-projection
    from gathered kv latents; causal flash attention (k-major scores, exp on
    ScalarE over chunk pairs, VectorE softmax denominator accumulation,
    software-pipelined emission so the PE never waits on exp).
  output: two AllToAlls (even heads overlap the odd-head attention; odd heads
    overlap the even half of the token-parallel output projection).

DMA queues: sync = input loads (+ phase-2/3 receives), vector = stores,
scalar = q receives + w_o prefetch. A tiny dummy AllGather at kernel start
eats the ~11us first-collective ncfw cold-start penalty.

bf16 matmuls, fp32 PSUM accumulation + softmax statistics, fp32 output.
"""
import sys
from contextlib import ExitStack

sys.path.insert(0, "/opt/trn_rl_repo")

import numpy as np
import ml_dtypes

import concourse.bacc as bacc
import concourse.mybir as mybir
import concourse.tile as tile
from concourse.bass_utils import run_bass_kernel_spmd

# ---- problem sizes (hardcoded per spec) ----
HID = 2048; H = 16; QLR = 1536; KVLR = 512
DN = 128; DR = 64; DV = 128; DQ = DN + DR
B = 2; S = 2048
THETA = 10000.0; EPS = 1e-6

NCORES = 8
T = B * S              # 4096 flattened tokens
TPC = T // NCORES      # 512 tokens per core
HPC = H // NCORES      # 2 heads per core
P = 128
NHID = HID // P        # 16
NQLR = QLR // P        # 12
CKW = KVLR + DR        # 576
QT_PER_B = S // 512    # 4 q-tiles of 512 per (b,h) unit
KB_PER_B = S // P      # 16 k-chunks of 128 per batch
WKK = HPC * DN         # 256

BF16 = mybir.dt.bfloat16
F32 = mybir.dt.float32
AF = mybir.ActivationFunctionType

_NC_CACHE = None


def _rope_dual(nc, pool, out_bf16, ps, cos_sb, sin_sb, tag):
    """RoPE on a [128, W] psum holding two 64-row head groups; writes bf16."""
    W = 512
    HDR = DR // 2
    rot = pool.tile([P, W], F32, tag=f"{tag}rot", name=f"{tag}rot", bufs=2)
    for g in range(2):
        o = g * DR
        nc.scalar.mul(rot[o:o + HDR, :], ps[o + HDR:o + DR, :], -1.0)
        nc.scalar.copy(rot[o + HDR:o + DR, :], ps[o:o + HDR, :])
    t1 = pool.tile([P, W], F32, tag=f"{tag}t1", name=f"{tag}t1", bufs=2)
    nc.vector.tensor_mul(t1[:], ps[:], cos_sb[:])
    nc.vector.tensor_mul(rot[:], rot[:], sin_sb[:])
    nc.vector.tensor_add(out_bf16[:], t1[:], rot[:])


def _phase1ab(nc, tc, ps1, ps1s, ps1b, hidT, wqaT, wkvaT, cosd, sind,
              latkv_in, latkv_all, cqn_sb, ones_col, ones_row, eps_t,
              cos_sb, sin_sb, RG, wqb_dma):
    """ckv path (+ kv AllGather) then cq path; SBUF freed on exit."""
    with tc.tile_pool(name="p1a", bufs=1) as p1a, \
         tc.tile_pool(name="p1t", bufs=2) as p1t, \
         tc.tile_pool(name="p1n", bufs=1) as p1n:
        hid_ch = [p1a.tile([P, TPC], BF16, tag=f"hid{kc}", name=f"hid{kc}")
                  for kc in range(NHID)]
        wkva_ch = [p1a.tile([P, CKW], BF16, tag=f"wkva{kc}", name=f"wkva{kc}")
                   for kc in range(NHID)]
        wqa_ch = [p1a.tile([P, QLR], BF16, tag=f"wqa{kc}", name=f"wqa{kc}")
                  for kc in range(NHID)]
        for kc in range(NHID):
            nc.sync.dma_start(hid_ch[kc][:], hidT.ap()[kc * P:(kc + 1) * P, :])
            nc.sync.dma_start(wkva_ch[kc][:], wkvaT.ap()[kc * P:(kc + 1) * P, :])
        nc.sync.dma_start(cos_sb[:], cosd.ap()[:])
        nc.sync.dma_start(sin_sb[:], sind.ap()[:])
        for kc in range(NHID):
            nc.sync.dma_start(wqa_ch[kc][:], wqaT.ap()[kc * P:(kc + 1) * P, :])
        wqb_dma()

        # --- ckv joint (kc-outer: 5 open psum groups, compute starts on
        #     the first arriving chunk) ---
        with tc.tile_pool(name="p1ckv", bufs=1) as p1ckv:
            ckv_f32 = p1ckv.tile([P, 4 * TPC], BF16)
            ssq_kv = ps1s.tile([1, TPC], F32)
            ps_m = [ps1.tile([P, TPC], F32, tag="proj", name=f"ckv{m}")
                    for m in range(4)]
            ps_pe = ps1.tile([DR, TPC], F32, tag="pe", bufs=1)
            for kc in range(NHID):
                for m in range(4):
                    nc.tensor.matmul(ps_m[m][:], wkva_ch[kc][:, m * P:(m + 1) * P],
                                     hid_ch[kc][:],
                                     start=(kc == 0), stop=(kc == NHID - 1))
                nc.tensor.matmul(ps_pe[:], wkva_ch[kc][:, KVLR:CKW],
                                 hid_ch[kc][:],
                                 start=(kc == 0), stop=(kc == NHID - 1))
            for m in range(4):
                nc.scalar.copy(ckv_f32[:, m * TPC:(m + 1) * TPC], ps_m[m][:])
                sq = p1t.tile([P, TPC], BF16, tag="sq")
                nc.vector.tensor_mul(sq[:], ckv_f32[:, m * TPC:(m + 1) * TPC],
                                     ckv_f32[:, m * TPC:(m + 1) * TPC])
                nc.tensor.matmul(ssq_kv[:], ones_col[:], sq[:],
                                 start=(m == 0), stop=(m == 3),
                                 skip_group_check=True)

            # k_pe rope (shared across heads)
            HDR = DR // 2
            rot = p1t.tile([DR, TPC], F32, tag="rot")
            nc.scalar.mul(rot[0:HDR, :], ps_pe[HDR:DR, :], -1.0)
            nc.scalar.copy(rot[HDR:DR, :], ps_pe[0:HDR, :])
            t1 = p1t.tile([DR, TPC], F32, tag="t1")
            nc.vector.tensor_mul(t1[:], ps_pe[:], cos_sb[0:DR, :])
            nc.vector.tensor_mul(rot[:], rot[:], sin_sb[0:DR, :])
            pe_out = p1t.tile([DR, TPC], BF16, tag="peo")
            nc.vector.tensor_add(pe_out[:], t1[:], rot[:])
            nc.scalar.dma_start(latkv_in[KVLR:CKW, :], pe_out[:])

            kv_norm = p1n.tile([1, TPC], F32, tag="nrm")
            nc.scalar.activation(kv_norm[:], ssq_kv[:], AF.Sqrt, bias=eps_t[:],
                                 scale=1.0 / KVLR)
            rn_kv = p1n.tile([1, TPC], F32, tag="rn")
            nc.vector.reciprocal_approx_fast(rn_kv[:], kv_norm[:])
            bkv = ps1b.tile([P, TPC], F32, tag="bc")
            nc.tensor.matmul(bkv[:], ones_row[:], rn_kv[:], start=True, stop=True)
            for m in range(4):
                lat_sb = p1t.tile([P, TPC], BF16, tag="sq")
                nc.vector.tensor_mul(lat_sb[:], ckv_f32[:, m * TPC:(m + 1) * TPC], bkv[:])
                nc.scalar.dma_start(latkv_in[m * P:(m + 1) * P, :], lat_sb[:])

            nc.gpsimd.collective_compute(
                "AllGather", mybir.AluOpType.bypass, replica_groups=RG,
                ins=[latkv_in.opt()], outs=[latkv_all.opt()])

        # --- cq (bf16 storage) + rmsnorm ---
        cq_bf = p1a.tile([P, NQLR * TPC], BF16)
        ssq_q = ps1s.tile([1, TPC], F32)
        for m in range(NQLR):
            ps = ps1.tile([P, TPC], F32, tag="proj", name="cqp")
            for kc in range(NHID):
                nc.tensor.matmul(ps[:], wqa_ch[kc][:, m * P:(m + 1) * P],
                                 hid_ch[kc][:],
                                 start=(kc == 0), stop=(kc == NHID - 1))
            nc.scalar.copy(cq_bf[:, m * TPC:(m + 1) * TPC], ps[:])
            sq = p1t.tile([P, TPC], BF16, tag="sq")
            nc.vector.tensor_mul(sq[:], cq_bf[:, m * TPC:(m + 1) * TPC],
                                 cq_bf[:, m * TPC:(m + 1) * TPC])
            nc.tensor.matmul(ssq_q[:], ones_col[:], sq[:],
                             start=(m == 0), stop=(m == NQLR - 1),
                             skip_group_check=True)
        sq_norm = p1n.tile([1, TPC], F32, tag="nrm")
        nc.scalar.activation(sq_norm[:], ssq_q[:], AF.Sqrt, bias=eps_t[:],
                             scale=1.0 / QLR)
        rn_q = p1n.tile([1, TPC], F32, tag="rn")
        nc.vector.reciprocal_approx_fast(rn_q[:], sq_norm[:])
        bq = ps1b.tile([P, TPC], F32, tag="bc")
        nc.tensor.matmul(bq[:], ones_row[:], rn_q[:], start=True, stop=True)
        for m in range(NQLR):
            nc.vector.tensor_mul(cqn_sb[:, m * TPC:(m + 1) * TPC],
                                 cq_bf[:, m * TPC:(m + 1) * TPC], bq[:])


def build_nc():
    nc = bacc.Bacc(None, target_bir_lowering=False, debug=False, num_devices=NCORES)

    # ---- per-core external inputs ----
    hidT = nc.dram_tensor("hidT", [HID, TPC], BF16, kind="ExternalInput")
    wqaT = nc.dram_tensor("wqaT", [HID, QLR], BF16, kind="ExternalInput")
    wkvaT = nc.dram_tensor("wkvaT", [HID, CKW], BF16, kind="ExternalInput")
    wqbT = nc.dram_tensor("wqbT", [QLR, H * DQ], BF16, kind="ExternalInput")
    wkvbkT = nc.dram_tensor("wkvbkT", [KVLR, HPC * DN], BF16, kind="ExternalInput")
    wkvbvT = nc.dram_tensor("wkvbvT", [KVLR, HPC * DV], BF16, kind="ExternalInput")
    woT = nc.dram_tensor("woT", [H * DV, HID], BF16, kind="ExternalInput")
    cosd = nc.dram_tensor("cosd", [P, TPC], F32, kind="ExternalInput")
    sind = nc.dram_tensor("sind", [P, TPC], F32, kind="ExternalInput")
    masks = nc.dram_tensor("masks", [P, P], BF16, kind="ExternalInput")
    trid = nc.dram_tensor("trid", [P, P], BF16, kind="ExternalInput")
    outT = nc.dram_tensor("outT", [HID, TPC], F32, kind="ExternalOutput")

    RG = [list(range(NCORES))]

    with tile.TileContext(nc) as tc:
        with tc.tile_pool(name="dram", bufs=1, space="DRAM") as dram, \
             tc.tile_pool(name="const", bufs=1) as const:
            latkv_in = dram.tile([CKW, TPC], BF16)
            latkv_all = dram.tile([NCORES * CKW, TPC], BF16, addr_space="Shared")
            qa_in = dram.tile([NCORES * 2 * P, TPC], BF16)   # [pe2|h0n] per pair
            qa_out = dram.tile([NCORES * 2 * P, TPC], BF16)
            qb_in = dram.tile([NCORES * P, TPC], BF16)       # h1n per pair
            qb_out = dram.tile([NCORES * P, TPC], BF16)
            oa_in = dram.tile([NCORES * DV, TPC], BF16)      # even heads out
            oa_out = dram.tile([NCORES * DV, TPC], BF16)
            ob_in = dram.tile([NCORES * DV, TPC], BF16)      # odd heads out
            ob_out = dram.tile([NCORES * DV, TPC], BF16)
            warm_in = dram.tile([1, 64], BF16)
            warm_out = dram.tile([NCORES, 64], BF16, addr_space="Shared")

            ones_col = const.tile([P, 1], BF16)
            nc.vector.memset(ones_col[:], 1.0)
            ones_row = const.tile([1, P], F32)
            nc.vector.memset(ones_row[:], 1.0)
            ones_row_bf = const.tile([1, P], BF16)
            nc.vector.memset(ones_row_bf[:], 1.0)
            ones_col_f = const.tile([P, 1], F32)
            nc.vector.memset(ones_col_f[:], 1.0)
            eps_t = const.tile([1, 1], F32)
            nc.vector.memset(eps_t[:], EPS)
            cos_sb = const.tile([P, TPC], F32)
            sin_sb = const.tile([P, TPC], F32)

            # warmup collective: eats the ~11us ncfw first-collective cold
            # start while phase-1a computes.
            warm_sb = const.tile([1, 64], BF16)
            nc.vector.memset(warm_sb[:], 0.0)
            nc.scalar.dma_start(warm_in[:], warm_sb[:])
            nc.gpsimd.collective_compute(
                "AllGather", mybir.AluOpType.bypass, replica_groups=RG,
                ins=[warm_in.opt()], outs=[warm_out.opt()])

            # ============ Phase 1: token-parallel compute ============
            p1q_stack = ExitStack()
            p1q = p1q_stack.enter_context(tc.tile_pool(name="p1q", bufs=1))
            ps1_stack = ExitStack()
            ps1 = ps1_stack.enter_context(tc.tile_pool(name="ps1", bufs=4, space="PSUM"))
            ps1s = ps1_stack.enter_context(tc.tile_pool(name="ps1s", bufs=1, space="PSUM"))
            ps1b = ps1_stack.enter_context(tc.tile_pool(name="ps1b", bufs=1, space="PSUM"))
            WQB = H * DQ  # 3072
            wqb_ch = [p1q.tile([P, WQB], BF16, tag=f"wqb{kc}", name=f"wqb{kc}")
                      for kc in range(NQLR)]

            def wqb_dma():
                for kc in range(NQLR):
                    nc.sync.dma_start(wqb_ch[kc][:], wqbT.ap()[kc * P:(kc + 1) * P, :])
            cqn_sb = p1q.tile([P, NQLR * TPC], BF16)
            # ---- phase 1a/1b scope (freed before q up-projection) ----
            _phase1ab(nc, tc, ps1, ps1s, ps1b, hidT, wqaT, wkvaT, cosd, sind,
                      latkv_in, latkv_all, cqn_sb, ones_col, ones_row,
                      eps_t, cos_sb, sin_sb, RG, wqb_dma)

            # ---- phase-2 receive issuance (sync queue, runs during 1c) ----
            # Long-lived attention pools go on the RIGHT side of SBUF so the
            # phase-1 pools (left) can release under them in stack order.
            att_stack = ExitStack()
            att_a = att_stack.enter_context(
                tc.tile_pool(name="att_a", bufs=1, side="right"))
            att2 = att_stack.enter_context(
                tc.tile_pool(name="att2", bufs=1, side="right"))
            p2w = tc.alloc_tile_pool(name="p2w", bufs=1, side="right")
            p2a = tc.alloc_tile_pool(name="p2a", bufs=1, side="right")
            kpe2 = att_a.tile([P, T], BF16)    # k_pe duplicated rows
            mask_sb = att_a.tile([P, P], BF16)  # additive -3e4 corner band
            tri_sb = att_a.tile([P, P], BF16)
            nc.sync.dma_start(tri_sb[:], trid.ap()[:])
            knope = att2.tile([P, 2 * T], BF16)
            v_sb = att2.tile([P, (T // P) * WKK], BF16)
            qnope = att2.tile([P, 2 * T], BF16)
            qpe = att2.tile([P, T], BF16)      # rows 0-63 h0, 64-127 h1
            wkk_sb = p2w.tile([P, 4 * WKK], BF16)
            wkv_sb = p2w.tile([P, 4 * WKK], BF16)
            for kc in range(4):
                nc.sync.dma_start(wkk_sb[:, kc * WKK:(kc + 1) * WKK],
                                  wkvbkT.ap()[kc * P:(kc + 1) * P, :])
                nc.sync.dma_start(wkv_sb[:, kc * WKK:(kc + 1) * WKK],
                                  wkvbvT.ap()[kc * P:(kc + 1) * P, :])
            nc.sync.dma_start(mask_sb[:], masks.ap()[:])
            ckv_js = []
            for j in range(NCORES):
                basek = j * CKW
                ckv_j = p2a.tile([P, 4 * TPC], BF16, tag=f"ckvj{j}",
                                 name=f"ckvj{j}")
                ckv_js.append(ckv_j)
                for r in range(4):
                    nc.sync.dma_start(ckv_j[:, r * TPC:(r + 1) * TPC],
                                      latkv_all[basek + r * P: basek + (r + 1) * P, :])
                nc.sync.dma_start(kpe2[0:DR, j * TPC:(j + 1) * TPC],
                                  latkv_all[basek + KVLR: basek + CKW, :])
                nc.sync.dma_start(kpe2[DR:P, j * TPC:(j + 1) * TPC],
                                  latkv_all[basek + KVLR: basek + CKW, :])

            # ============ Phase 1c: q up-projection for ALL heads ============
            with tc.tile_pool(name="p1qt", bufs=3) as p1qt:
                for mb in range(16):
                    ps = ps1.tile([P, TPC], F32, tag="proj", name="qup")
                    for kc in range(NQLR):
                        nc.tensor.matmul(ps[:], wqb_ch[kc][:, mb * P:(mb + 1) * P],
                                         cqn_sb[:, kc * TPC:(kc + 1) * TPC],
                                         start=(kc == 0), stop=(kc == NQLR - 1))
                    qo = p1qt.tile([P, TPC], BF16, tag="qo")
                    if mb % 2 == 0:  # pe2 block -> rope
                        _rope_dual(nc, p1qt, qo, ps, cos_sb, sin_sb, "q")
                    else:
                        nc.scalar.copy(qo[:], ps[:])
                    nc.scalar.dma_start(qa_in[mb * P:(mb + 1) * P, :], qo[:])
                nc.gpsimd.collective_compute(
                    "AllToAll", mybir.AluOpType.bypass, replica_groups=RG,
                    ins=[qa_in.opt()], outs=[qa_out.opt()])
                for mb in range(8):
                    ps = ps1.tile([P, TPC], F32, tag="proj", name="qup")
                    for kc in range(NQLR):
                        nc.tensor.matmul(ps[:], wqb_ch[kc][:, (16 + mb) * P:(17 + mb) * P],
                                         cqn_sb[:, kc * TPC:(kc + 1) * TPC],
                                         start=(kc == 0), stop=(kc == NQLR - 1))
                    qo = p1qt.tile([P, TPC], BF16, tag="qo")
                    nc.scalar.copy(qo[:], ps[:])
                    nc.scalar.dma_start(qb_in[mb * P:(mb + 1) * P, :], qo[:])
                nc.gpsimd.collective_compute(
                    "AllToAll", mybir.AluOpType.bypass, replica_groups=RG,
                    ins=[qb_in.opt()], outs=[qb_out.opt()])
            ps1_stack.close()
            p1q_stack.close()

            # ===== Phase 2: q receive + k/v up-proj (overlaps q AllToAlls) =====
            # q receives on the sync queue (idle after the phase-1 loads)
            for i in range(NCORES):
                nc.sync.dma_start(qpe[:, i * TPC:(i + 1) * TPC],
                                  qa_out[i * 2 * P: i * 2 * P + P, :])
                nc.sync.dma_start(qnope[:, i * TPC:(i + 1) * TPC],
                                  qa_out[i * 2 * P + P: (i + 1) * 2 * P, :])
            for i in range(NCORES):
                nc.sync.dma_start(qnope[:, T + i * TPC: T + (i + 1) * TPC],
                                  qb_out[i * P:(i + 1) * P, :])

            # phase-3 even-half prefetch (scalar queue; runs during kv-up)
            p3w = tc.alloc_tile_pool(name="p3w", bufs=1, side="left")
            woe_sb = p3w.tile([P, NCORES * HID], BF16)
            oe_sb = p3w.tile([P, NCORES * TPC], BF16)
            for i in range(NCORES):
                nc.scalar.dma_start(woe_sb[:, i * HID:(i + 1) * HID],
                                    woT.ap()[(2 * i) * P:(2 * i + 1) * P, :])

            with tc.tile_pool(name="ps2", bufs=4, space="PSUM") as ps2:
                # knope h0 first (unblocks even-head attention), then v for
                # both heads, then knope h1.
                for m in range(HPC):
                    for j in range(NCORES):
                        ps = ps2.tile([P, TPC], F32, tag="proj")
                        for kc in range(4):
                            nc.tensor.matmul(
                                ps[:], wkk_sb[:, kc * WKK + m * P: kc * WKK + (m + 1) * P],
                                ckv_js[j][:, kc * TPC:(kc + 1) * TPC],
                                start=(kc == 0), stop=(kc == 3))
                        nc.vector.tensor_copy(knope[:, m * T + j * TPC: m * T + (j + 1) * TPC], ps[:])
                    if m == 0:
                        for j in range(NCORES):
                            for tb in range(TPC // P):
                                ps = ps2.tile([P, WKK], F32, tag="proj")
                                for kc in range(4):
                                    nc.tensor.matmul(
                                        ps[:], ckv_js[j][:, kc * TPC + tb * P: kc * TPC + (tb + 1) * P],
                                        wkv_sb[:, kc * WKK:(kc + 1) * WKK],
                                        start=(kc == 0), stop=(kc == 3))
                                jb = j * (TPC // P) + tb
                                nc.vector.tensor_copy(v_sb[:, jb * WKK:(jb + 1) * WKK], ps[:])
            p2a.release()
            p2w.release()

            # ============ attention (4 causal units, software-pipelined) ============
            # chunk-granular: per 128-key chunk, 2 score matmuls -> exp
            # (ScalarE) -> mask/denominator (VectorE) -> v matmul. v matmuls
            # lag 2 chunks and tails lag a tile so PE-queue waits are
            # pre-satisfied and LDWEIGHTS prefetch is never blocked.
            from collections import deque
            with tc.tile_pool(name="att_t", bufs=4, side="right") as att_t, \
                 tc.tile_pool(name="att_d", bufs=2, side="right") as att_d, \
                 tc.tile_pool(name="ps_s", bufs=4, space="PSUM") as ps_s_pool, \
                 tc.tile_pool(name="ps_o", bufs=2, space="PSUM") as ps_o_pool, \
                 tc.tile_pool(name="ps_d", bufs=1, space="PSUM") as ps_d_pool:
                pend_v = deque()
                tails = deque()

                def flush_v(keep):
                    while len(pend_v) > keep:
                        pend_v.popleft()()

                for u in range(4):  # hl-major: (hl, bb)
                    hl, bb = u // 2, u % 2
                    for qt in range(QT_PER_B):
                        qoff = bb * S + qt * 512
                        ps_o = ps_o_pool.tile([P, 512], F32, tag="pso")
                        acc = att_d.tile([P, 512], F32, tag="acc")
                        nkc = 4 * (qt + 1)
                        for kc in range(nkc):
                            koff = bb * S + kc * P
                            # diagonal chunks: only queries >= mi*128 can
                            # attend these keys; narrow all work to [o0:512)
                            mi = kc - 4 * qt
                            o0 = mi * P if mi > 0 else 0
                            diag = mi >= 0
                            ps_sc = ps_s_pool.tile([P, 512], F32, tag="pss")
                            nc.tensor.matmul(
                                ps_sc[:, o0:512],
                                knope[:, hl * T + koff: hl * T + koff + P],
                                qnope[:, hl * T + qoff + o0: hl * T + qoff + 512],
                                start=True, stop=False)
                            nc.tensor.matmul(
                                ps_sc[:, o0:512],
                                kpe2[hl * DR: hl * DR + DR, koff: koff + P],
                                qpe[hl * DR: hl * DR + DR, qoff + o0: qoff + 512],
                                start=False, stop=not diag)
                            if diag:
                                # additive causal corner: tri^T @ band puts
                                # -3e4 on q-o0 < k within the corner block
                                nc.tensor.matmul(
                                    ps_sc[:, o0:o0 + P], tri_sb[:], mask_sb[:],
                                    start=False, stop=True)
                            # stagger: older v matmuls + one deferred tail
                            # step go behind this chunk's score matmuls.
                            # Tails pop only from chunk 2 on, AFTER the
                            # previous tile's last v matmuls have been
                            # emitted (ps_o must be complete before the
                            # tail's ou copy reads it).
                            flush_v(2)
                            if kc >= 2 and tails:
                                tails.popleft()()
                            ex = att_t.tile([P, 512], BF16, tag="ex")
                            nc.scalar.activation(ex[:, o0:512], ps_sc[:, o0:512],
                                                 AF.Exp)
                            if kc == 0:
                                nc.vector.tensor_copy(acc[:], ex[:])
                            else:
                                nc.vector.tensor_add(acc[:, o0:512],
                                                     acc[:, o0:512],
                                                     ex[:, o0:512])

                            def emit_v(ps_o=ps_o, ex=ex, kc=kc, bb=bb, hl=hl,
                                       nkc=nkc, o0=o0):
                                jb = bb * KB_PER_B + kc
                                nc.tensor.matmul(
                                    ps_o[:, o0:512],
                                    v_sb[:, jb * WKK + hl * DV: jb * WKK + (hl + 1) * DV],
                                    ex[:, o0:512],
                                    start=(kc == 0), stop=(kc == nkc - 1),
                                    skip_group_check=True)
                            pend_v.append(emit_v)

                        # deferred tail steps (emitted behind the next tile's
                        # first two score pairs, via the FIFO)
                        cell = {}

                        def emit_t1(acc=acc, cell=cell):
                            # cast the f32 accumulator to bf16 so the ones
                            # matmul streams at 1 cycle/col instead of 4
                            acc_bf = att_t.tile([P, 512], BF16, tag="accb",
                                                bufs=2)
                            nc.vector.tensor_copy(acc_bf[:], acc[:])
                            ps_d = ps_d_pool.tile([1, 512], F32, tag="psd", bufs=1)
                            nc.tensor.matmul(ps_d[:], ones_col[:], acc_bf[:],
                                             start=True, stop=True)
                            recip = att_t.tile([1, 512], F32, tag="rcp")
                            nc.vector.reciprocal_approx_fast(recip[:], ps_d[:])
                            recip_bf = att_t.tile([1, 512], BF16, tag="rcpb")
                            nc.vector.tensor_copy(recip_bf[:], recip[:])
                            cell['r'] = recip_bf

                        def emit_t2(ps_o=ps_o, hl=hl, bb=bb, qt=qt, cell=cell):
                            recip_bf = cell['r']
                            bc = ps_d_pool.tile([P, 512], F32, tag="psd", bufs=1)
                            nc.tensor.matmul(bc[:], ones_row_bf[:], recip_bf[:],
                                             start=True, stop=True)
                            ou = att_t.tile([P, 512], F32, tag="ou", bufs=2)
                            nc.vector.tensor_copy(ou[:], ps_o[:])
                            on = att_t.tile([P, 512], BF16, tag="on")
                            nc.vector.tensor_mul(on[:], ou[:], bc[:])
                            blk = bb * QT_PER_B + qt
                            tgt = oa_in if hl == 0 else ob_in
                            nc.scalar.dma_start(tgt[blk * DV:(blk + 1) * DV, :], on[:])
                            if hl == 0 and bb == 1 and qt == QT_PER_B - 1:
                                nc.gpsimd.collective_compute(
                                    "AllToAll", mybir.AluOpType.bypass,
                                    replica_groups=RG,
                                    ins=[oa_in.opt()], outs=[oa_out.opt()])
                                # oe receives MUST be emitted after the
                                # collective so the dep tracker sees the
                                # oa_out writer (sync queue, idle here)
                                for i in range(NCORES):
                                    nc.sync.dma_start(
                                        oe_sb[:, i * TPC:(i + 1) * TPC],
                                        oa_out[i * P:(i + 1) * P, :])
                        tails.append(emit_t1)
                        tails.append(emit_t2)
                # end of stream: flush remaining
                flush_v(0)
                while tails:
                    tails.popleft()()

            att_stack.close()
            nc.gpsimd.collective_compute(
                "AllToAll", mybir.AluOpType.bypass, replica_groups=RG,
                ins=[ob_in.opt()], outs=[ob_out.opt()])

            # ============ Phase 3: two passes (pass 1 overlaps the ob AllToAll) ============
            with tc.tile_pool(name="p3t", bufs=3) as p3t, \
                 tc.tile_pool(name="ps3", bufs=4, space="PSUM") as ps3:
                part_sb = p3t.tile([P, NHID * TPC], F32, tag="part", bufs=1)
                woo_sb = p3t.tile([P, NCORES * HID], BF16, tag="woo", bufs=1)
                oo_sb = p3t.tile([P, NCORES * TPC], BF16, tag="oo", bufs=1)
                for i in range(NCORES):
                    nc.sync.dma_start(woo_sb[:, i * HID:(i + 1) * HID],
                                      woT.ap()[(2 * i + 1) * P:(2 * i + 2) * P, :])
                for i in range(NCORES):
                    nc.sync.dma_start(oo_sb[:, i * TPC:(i + 1) * TPC],
                                      ob_out[i * P:(i + 1) * P, :])
                for m in range(NHID):
                    ps = ps3.tile([P, TPC], F32, tag="proj")
                    for i in range(NCORES):
                        nc.tensor.matmul(
                            ps[:], woe_sb[:, i * HID + m * P: i * HID + (m + 1) * P],
                            oe_sb[:, i * TPC:(i + 1) * TPC],
                            start=(i == 0), stop=(i == NCORES - 1))
                    nc.scalar.copy(part_sb[:, m * TPC:(m + 1) * TPC], ps[:])
                for m in range(NHID):
                    ps = ps3.tile([P, TPC], F32, tag="proj")
                    for i in range(NCORES):
                        nc.tensor.matmul(
                            ps[:], woo_sb[:, i * HID + m * P: i * HID + (m + 1) * P],
                            oo_sb[:, i * TPC:(i + 1) * TPC],
                            start=(i == 0), stop=(i == NCORES - 1))
                    ot = p3t.tile([P, TPC], F32, tag="ot")
                    nc.vector.tensor_add(ot[:], ps[:], part_sb[:, m * TPC:(m + 1) * TPC])
                    nc.sync.dma_start(outT.ap()[m * P:(m + 1) * P, :], ot[:])
            p3w.release()
    nc.finalize()
    return nc


def _bf16(x):
    return np.ascontiguousarray(x.astype(ml_dtypes.bfloat16))


def _rope_tables():
    inv_freq = 1.0 / (THETA ** (np.arange(0, DR, 2, dtype=np.float64) / DR))
    t = np.arange(S, dtype=np.float64)
    freqs = np.outer(t, inv_freq)
    emb = np.concatenate((freqs, freqs), axis=-1)
    return np.cos(emb).astype(np.float32), np.sin(emb).astype(np.float32)


def prepare_inputs(hidden_states, w_qa, q_a_ln_w, w_qb, w_kva, kv_a_ln_w, w_kvb, w_o):
    hidden_states = np.asarray(hidden_states, dtype=np.float32)
    w_qa = np.asarray(w_qa, dtype=np.float32)
    q_a_ln_w = np.asarray(q_a_ln_w, dtype=np.float32)
    w_qb = np.asarray(w_qb, dtype=np.float32)
    w_kva = np.asarray(w_kva, dtype=np.float32)
    kv_a_ln_w = np.asarray(kv_a_ln_w, dtype=np.float32)
    w_kvb = np.asarray(w_kvb, dtype=np.float32)
    w_o = np.asarray(w_o, dtype=np.float32)

    flat = hidden_states.reshape(T, HID)
    cos, sin = _rope_tables()          # [S, DR]
    scale = DQ ** -0.5

    pos = np.arange(T) % S
    cos_d = cos[pos].T                 # [DR, T]
    sin_d = sin[pos].T

    # additive causal corner band: out[k,c] = sum_{j<=k} band[j,c] = -3e4
    # iff c < k (tri is lower-cumulative along contraction)
    BIG = -30000.0
    band = np.zeros((P, P), np.float32)
    for c in range(P - 1):
        band[c + 1, c] = BIG
    masks = _bf16(band)
    jj = np.arange(P)[:, None]
    kk = np.arange(P)[None, :]
    tri = _bf16((kk >= jj).astype(np.float32))

    w_qb_eff = (w_qb * q_a_ln_w[None, :]) * scale       # [H*DQ, QLR]
    w_kvb_eff = w_kvb * kv_a_ln_w[None, :]              # [H*(DN+DV), KVLR]

    # w_qb rows permuted: block A = per pair j [h0 pe | h1 pe | h0 nope],
    # block B = per pair j [h1 nope]
    rows = []
    for j in range(NCORES):
        h0, h1 = 2 * j, 2 * j + 1
        rows.append(w_qb_eff[h0 * DQ + DN: h0 * DQ + DQ])   # h0 pe (64)
        rows.append(w_qb_eff[h1 * DQ + DN: h1 * DQ + DQ])   # h1 pe (64)
        rows.append(w_qb_eff[h0 * DQ: h0 * DQ + DN])        # h0 nope (128)
    for j in range(NCORES):
        h1 = 2 * j + 1
        rows.append(w_qb_eff[h1 * DQ: h1 * DQ + DN])        # h1 nope (128)
    wqbT_full = _bf16(np.concatenate(rows, axis=0).T)       # [QLR, 3072]

    wqaT = _bf16(w_qa.T)
    wkvaT = _bf16(w_kva.T)
    woT = _bf16(w_o.T)

    in_maps = []
    for c in range(NCORES):
        heads = [HPC * c + h for h in range(HPC)]
        krows = [w_kvb_eff[h * (DN + DV): h * (DN + DV) + DN] for h in heads]
        wkvbkT_c = _bf16(np.concatenate(krows, axis=0).T)
        vrows = [w_kvb_eff[h * (DN + DV) + DN: (h + 1) * (DN + DV)] for h in heads]
        wkvbvT_c = _bf16(np.concatenate(vrows, axis=0).T)

        tok0 = c * TPC
        cosl = cos_d[:, tok0:tok0 + TPC]
        sinl = sin_d[:, tok0:tok0 + TPC]
        in_maps.append({
            "hidT": _bf16(flat[tok0:tok0 + TPC].T),
            "wqaT": wqaT, "wkvaT": wkvaT,
            "wqbT": wqbT_full, "wkvbkT": wkvbkT_c, "wkvbvT": wkvbvT_c,
            "woT": woT,
            "cosd": np.ascontiguousarray(np.concatenate([cosl, cosl], axis=0)),
            "sind": np.ascontiguousarray(np.concatenate([sinl, sinl], axis=0)),
            "masks": masks,
            "trid": tri,
        })
    return in_maps


def kernel(hidden_states, w_qa, q_a_ln_w, w_qb, w_kva, kv_a_ln_w, w_kvb, w_o,
           _trace=False):
    global _NC_CACHE
    if _NC_CACHE is None:
        _NC_CACHE = build_nc()
    nc = _NC_CACHE
    in_maps = prepare_inputs(hidden_states, w_qa, q_a_ln_w, w_qb, w_kva,
                             kv_a_ln_w, w_kvb, w_o)
    res = run_bass_kernel_spmd(nc, in_maps, core_ids=list(range(NCORES)),
                               trace=_trace)
    out = np.empty((T, HID), dtype=np.float32)
    for c in range(NCORES):
        out[c * TPC:(c + 1) * TPC] = res.results[c]["outT"].T
    if _trace:
        kernel._last_result = res
    return out.reshape(B, S, HID)


# revision 54
# speedup vs baseline: 1.0511x; 1.0511x over previous
"""MLA (DeepSeek-style) attention block on 8 Trainium2 NeuronCores.

Sharding:
  phase 1 (token-parallel, 8 x 512 tokens): LoRA-A down-projections + rmsnorm
    + k_pe rope; small AllGather of the kv latents (576 dims); q up-projection
    for ALL 16 heads on the token side + rope, shipped to head shards via two
    shard-aligned AllToAlls (pe+even-nope first, odd-nope second).
  phase 2 (head-parallel, 2 heads x 2 batches per core): k/v up-projection
    from gathered kv latents; causal flash attention (k-major scores, exp on
    ScalarE over chunk pairs, VectorE softmax denominator accumulation,
    software-pipelined emission so the PE never waits on exp).
  output: two AllToAlls (even heads overlap the odd-head attention; odd heads
    overlap the even half of the token-parallel output projection).

DMA queues: sync = input loads (+ phase-2/3 receives), vector = stores,
scalar = q receives + w_o prefetch. A tiny dummy AllGather at kernel start
eats the ~11us first-collective ncfw cold-start penalty.

bf16 matmuls, fp32 PSUM accumulation + softmax statistics, fp32 output.
"""
import sys
from contextlib import ExitStack

sys.path.insert(0, "/opt/trn_rl_repo")

import numpy as np
import ml_dtypes

import concourse.bacc as bacc
import concourse.mybir as mybir
import concourse.tile as tile
from concourse.bass_utils import run_bass_kernel_spmd

# ---- problem sizes (hardcoded per spec) ----
HID = 2048; H = 16; QLR = 1536; KVLR = 512
DN = 128; DR = 64; DV = 128; DQ = DN + DR
B = 2; S = 2048
THETA = 10000.0; EPS = 1e-6

NCORES = 8
T = B * S              # 4096 flattened tokens
TPC = T // NCORES      # 512 tokens per core
HPC = H // NCORES      # 2 heads per core
P = 128
NHID = HID // P        # 16
NQLR = QLR // P        # 12
CKW = KVLR + DR        # 576
QT_PER_B = S // 512    # 4 q-tiles of 512 per (b,h) unit
KB_PER_B = S // P      # 16 k-chunks of 128 per batch
WKK = HPC * DN         # 256

BF16 = mybir.dt.bfloat16
F32 = mybir.dt.float32
AF = mybir.ActivationFunctionType

_NC_CACHE = None


def _rope_dual(nc, pool, out_bf16, ps, cos_sb, sin_sb, tag):
    """RoPE on a [128, W] psum holding two 64-row head groups; writes bf16."""
    W = 512
    HDR = DR // 2
    rot = pool.tile([P, W], F32, tag=f"{tag}rot", name=f"{tag}rot", bufs=2)
    for g in range(2):
        o = g * DR
        nc.scalar.mul(rot[o:o + HDR, :], ps[o + HDR:o + DR, :], -1.0)
        nc.scalar.copy(rot[o + HDR:o + DR, :], ps[o:o + HDR, :])
    t1 = pool.tile([P, W], F32, tag=f"{tag}t1", name=f"{tag}t1", bufs=2)
    nc.vector.tensor_mul(t1[:], ps[:], cos_sb[:])
    nc.vector.tensor_mul(rot[:], rot[:], sin_sb[:])
    nc.vector.tensor_add(out_bf16[:], t1[:], rot[:])


def _phase1ab(nc, tc, ps1, ps1s, ps1b, hidT, wqaT, wkvaT, cosd, sind,
              latkv_in, latkv_all, cqn_sb, ones_col, ones_row, eps_t,
              cos_sb, sin_sb, RG, wqb_dma):
    """ckv path (+ kv AllGather) then cq path; SBUF freed on exit."""
    with tc.tile_pool(name="p1a", bufs=1) as p1a, \
         tc.tile_pool(name="p1t", bufs=2) as p1t, \
         tc.tile_pool(name="p1n", bufs=1) as p1n:
        hid_ch = [p1a.tile([P, TPC], BF16, tag=f"hid{kc}", name=f"hid{kc}")
                  for kc in range(NHID)]
        wkva_ch = [p1a.tile([P, CKW], BF16, tag=f"wkva{kc}", name=f"wkva{kc}")
                   for kc in range(NHID)]
        wqa_ch = [p1a.tile([P, QLR], BF16, tag=f"wqa{kc}", name=f"wqa{kc}")
                  for kc in range(NHID)]
        for kc in range(NHID):
            nc.sync.dma_start(hid_ch[kc][:], hidT.ap()[kc * P:(kc + 1) * P, :])
            nc.sync.dma_start(wkva_ch[kc][:], wkvaT.ap()[kc * P:(kc + 1) * P, :])
        nc.sync.dma_start(cos_sb[:], cosd.ap()[:])
        nc.sync.dma_start(sin_sb[:], sind.ap()[:])
        for kc in range(NHID):
            nc.sync.dma_start(wqa_ch[kc][:], wqaT.ap()[kc * P:(kc + 1) * P, :])
        wqb_dma()

        # --- ckv joint (kc-outer: 5 open psum groups, compute starts on
        #     the first arriving chunk) ---
        with tc.tile_pool(name="p1ckv", bufs=1) as p1ckv:
            ckv_f32 = p1ckv.tile([P, 4 * TPC], BF16)
            ssq_kv = ps1s.tile([1, TPC], F32)
            ps_m = [ps1.tile([P, TPC], F32, tag="proj", name=f"ckv{m}")
                    for m in range(4)]
            ps_pe = ps1.tile([DR, TPC], F32, tag="pe", bufs=1)
            for kc in range(NHID):
                for m in range(4):
                    nc.tensor.matmul(ps_m[m][:], wkva_ch[kc][:, m * P:(m + 1) * P],
                                     hid_ch[kc][:],
                                     start=(kc == 0), stop=(kc == NHID - 1))
                nc.tensor.matmul(ps_pe[:], wkva_ch[kc][:, KVLR:CKW],
                                 hid_ch[kc][:],
                                 start=(kc == 0), stop=(kc == NHID - 1))
            for m in range(4):
                nc.scalar.copy(ckv_f32[:, m * TPC:(m + 1) * TPC], ps_m[m][:])
                sq = p1t.tile([P, TPC], BF16, tag="sq")
                nc.vector.tensor_mul(sq[:], ckv_f32[:, m * TPC:(m + 1) * TPC],
                                     ckv_f32[:, m * TPC:(m + 1) * TPC])
                nc.tensor.matmul(ssq_kv[:], ones_col[:], sq[:],
                                 start=(m == 0), stop=(m == 3),
                                 skip_group_check=True)

            # k_pe rope (shared across heads)
            HDR = DR // 2
            rot = p1t.tile([DR, TPC], F32, tag="rot")
            nc.scalar.mul(rot[0:HDR, :], ps_pe[HDR:DR, :], -1.0)
            nc.scalar.copy(rot[HDR:DR, :], ps_pe[0:HDR, :])
            t1 = p1t.tile([DR, TPC], F32, tag="t1")
            nc.vector.tensor_mul(t1[:], ps_pe[:], cos_sb[0:DR, :])
            nc.vector.tensor_mul(rot[:], rot[:], sin_sb[0:DR, :])
            pe_out = p1t.tile([DR, TPC], BF16, tag="peo")
            nc.vector.tensor_add(pe_out[:], t1[:], rot[:])
            nc.scalar.dma_start(latkv_in[KVLR:CKW, :], pe_out[:])

            kv_norm = p1n.tile([1, TPC], F32, tag="nrm")
            nc.scalar.activation(kv_norm[:], ssq_kv[:], AF.Sqrt, bias=eps_t[:],
                                 scale=1.0 / KVLR)
            rn_kv = p1n.tile([1, TPC], F32, tag="rn")
            nc.vector.reciprocal_approx_fast(rn_kv[:], kv_norm[:])
            bkv = ps1b.tile([P, TPC], F32, tag="bc")
            nc.tensor.matmul(bkv[:], ones_row[:], rn_kv[:], start=True, stop=True)
            for m in range(4):
                lat_sb = p1t.tile([P, TPC], BF16, tag="sq")
                nc.vector.tensor_mul(lat_sb[:], ckv_f32[:, m * TPC:(m + 1) * TPC], bkv[:])
                nc.scalar.dma_start(latkv_in[m * P:(m + 1) * P, :], lat_sb[:])

            nc.gpsimd.collective_compute(
                "AllGather", mybir.AluOpType.bypass, replica_groups=RG,
                ins=[latkv_in.opt()], outs=[latkv_all.opt()])

        # --- cq (bf16 storage) + rmsnorm ---
        cq_bf = p1a.tile([P, NQLR * TPC], BF16)
        ssq_q = ps1s.tile([1, TPC], F32)
        for m in range(NQLR):
            ps = ps1.tile([P, TPC], F32, tag="proj", name="cqp")
            for kc in range(NHID):
                nc.tensor.matmul(ps[:], wqa_ch[kc][:, m * P:(m + 1) * P],
                                 hid_ch[kc][:],
                                 start=(kc == 0), stop=(kc == NHID - 1))
            nc.scalar.copy(cq_bf[:, m * TPC:(m + 1) * TPC], ps[:])
            sq = p1t.tile([P, TPC], BF16, tag="sq")
            nc.vector.tensor_mul(sq[:], cq_bf[:, m * TPC:(m + 1) * TPC],
                                 cq_bf[:, m * TPC:(m + 1) * TPC])
            nc.tensor.matmul(ssq_q[:], ones_col[:], sq[:],
                             start=(m == 0), stop=(m == NQLR - 1),
                             skip_group_check=True)
        sq_norm = p1n.tile([1, TPC], F32, tag="nrm")
        nc.scalar.activation(sq_norm[:], ssq_q[:], AF.Sqrt, bias=eps_t[:],
                             scale=1.0 / QLR)
        rn_q = p1n.tile([1, TPC], F32, tag="rn")
        nc.vector.reciprocal_approx_fast(rn_q[:], sq_norm[:])
        bq = ps1b.tile([P, TPC], F32, tag="bc")
        nc.tensor.matmul(bq[:], ones_row[:], rn_q[:], start=True, stop=True)
        for m in range(NQLR):
            nc.vector.tensor_mul(cqn_sb[:, m * TPC:(m + 1) * TPC],
                                 cq_bf[:, m * TPC:(m + 1) * TPC], bq[:])


def build_nc():
    nc = bacc.Bacc(None, target_bir_lowering=False, debug=False, num_devices=NCORES)

    # ---- per-core external inputs ----
    hidT = nc.dram_tensor("hidT", [HID, TPC], BF16, kind="ExternalInput")
    wqaT = nc.dram_tensor("wqaT", [HID, QLR], BF16, kind="ExternalInput")
    wkvaT = nc.dram_tensor("wkvaT", [HID, CKW], BF16, kind="ExternalInput")
    wqbT = nc.dram_tensor("wqbT", [QLR, H * DQ], BF16, kind="ExternalInput")
    wkvbkT = nc.dram_tensor("wkvbkT", [KVLR, HPC * DN], BF16, kind="ExternalInput")
    wkvbvT = nc.dram_tensor("wkvbvT", [KVLR, HPC * DV], BF16, kind="ExternalInput")
    woT = nc.dram_tensor("woT", [H * DV, HID], BF16, kind="ExternalInput")
    cosd = nc.dram_tensor("cosd", [P, TPC], F32, kind="ExternalInput")
    sind = nc.dram_tensor("sind", [P, TPC], F32, kind="ExternalInput")
    masks = nc.dram_tensor("masks", [P, P], BF16, kind="ExternalInput")
    trid = nc.dram_tensor("trid", [P, P], BF16, kind="ExternalInput")
    outT = nc.dram_tensor("outT", [HID, TPC], F32, kind="ExternalOutput")

    RG = [list(range(NCORES))]

    with tile.TileContext(nc) as tc:
        with tc.tile_pool(name="dram", bufs=1, space="DRAM") as dram, \
             tc.tile_pool(name="const", bufs=1) as const:
            latkv_in = dram.tile([CKW, TPC], BF16)
            latkv_all = dram.tile([NCORES * CKW, TPC], BF16, addr_space="Shared")
            qa_in = dram.tile([NCORES * 2 * P, TPC], BF16)   # [pe2|h0n] per pair
            qa_out = dram.tile([NCORES * 2 * P, TPC], BF16)
            qb_in = dram.tile([NCORES * P, TPC], BF16)       # h1n per pair
            qb_out = dram.tile([NCORES * P, TPC], BF16)
            oa_in = dram.tile([NCORES * DV, TPC], BF16)      # even heads out
            oa_out = dram.tile([NCORES * DV, TPC], BF16)
            ob_in = dram.tile([NCORES * DV, TPC], BF16)      # odd heads out
            ob_out = dram.tile([NCORES * DV, TPC], BF16)
            warm_in = dram.tile([1, 64], BF16)
            warm_out = dram.tile([NCORES, 64], BF16, addr_space="Shared")

            ones_col = const.tile([P, 1], BF16)
            nc.vector.memset(ones_col[:], 1.0)
            ones_row = const.tile([1, P], F32)
            nc.vector.memset(ones_row[:], 1.0)
            ones_row_bf = const.tile([1, P], BF16)
            nc.vector.memset(ones_row_bf[:], 1.0)
            ones_col_f = const.tile([P, 1], F32)
            nc.vector.memset(ones_col_f[:], 1.0)
            eps_t = const.tile([1, 1], F32)
            nc.vector.memset(eps_t[:], EPS)
            cos_sb = const.tile([P, TPC], F32)
            sin_sb = const.tile([P, TPC], F32)

            # warmup collective: eats the ~11us ncfw first-collective cold
            # start while phase-1a computes.
            warm_sb = const.tile([1, 64], BF16)
            nc.vector.memset(warm_sb[:], 0.0)
            nc.scalar.dma_start(warm_in[:], warm_sb[:])
            nc.gpsimd.collective_compute(
                "AllGather", mybir.AluOpType.bypass, replica_groups=RG,
                ins=[warm_in.opt()], outs=[warm_out.opt()])

            # ============ Phase 1: token-parallel compute ============
            p1q_stack = ExitStack()
            p1q = p1q_stack.enter_context(tc.tile_pool(name="p1q", bufs=1))
            ps1_stack = ExitStack()
            ps1 = ps1_stack.enter_context(tc.tile_pool(name="ps1", bufs=4, space="PSUM"))
            ps1s = ps1_stack.enter_context(tc.tile_pool(name="ps1s", bufs=1, space="PSUM"))
            ps1b = ps1_stack.enter_context(tc.tile_pool(name="ps1b", bufs=1, space="PSUM"))
            WQB = H * DQ  # 3072
            wqb_ch = [p1q.tile([P, WQB], BF16, tag=f"wqb{kc}", name=f"wqb{kc}")
                      for kc in range(NQLR)]

            def wqb_dma():
                for kc in range(NQLR):
                    nc.sync.dma_start(wqb_ch[kc][:], wqbT.ap()[kc * P:(kc + 1) * P, :])
            cqn_sb = p1q.tile([P, NQLR * TPC], BF16)
            # ---- phase 1a/1b scope (freed before q up-projection) ----
            _phase1ab(nc, tc, ps1, ps1s, ps1b, hidT, wqaT, wkvaT, cosd, sind,
                      latkv_in, latkv_all, cqn_sb, ones_col, ones_row,
                      eps_t, cos_sb, sin_sb, RG, wqb_dma)

            # ---- phase-2 receive issuance (sync queue, runs during 1c) ----
            # Long-lived attention pools go on the RIGHT side of SBUF so the
            # phase-1 pools (left) can release under them in stack order.
            att_stack = ExitStack()
            att_a = att_stack.enter_context(
                tc.tile_pool(name="att_a", bufs=1, side="right"))
            att2 = att_stack.enter_context(
                tc.tile_pool(name="att2", bufs=1, side="right"))
            p2w = tc.alloc_tile_pool(name="p2w", bufs=1, side="right")
            p2a = tc.alloc_tile_pool(name="p2a", bufs=1, side="right")
            kpe2 = att_a.tile([P, T], BF16)    # k_pe duplicated rows
            mask_sb = att_a.tile([P, P], BF16)  # additive -3e4 corner band
            tri_sb = att_a.tile([P, P], BF16)
            nc.sync.dma_start(tri_sb[:], trid.ap()[:])
            knope = att2.tile([P, 2 * T], BF16)
            v_sb = att2.tile([P, (T // P) * WKK], BF16)
            qnope = att2.tile([P, 2 * T], BF16)
            qpe = att2.tile([P, T], BF16)      # rows 0-63 h0, 64-127 h1
            wkk_sb = p2w.tile([P, 4 * WKK], BF16)
            wkv_sb = p2w.tile([P, 4 * WKK], BF16)
            for kc in range(4):
                nc.sync.dma_start(wkk_sb[:, kc * WKK:(kc + 1) * WKK],
                                  wkvbkT.ap()[kc * P:(kc + 1) * P, :])
                nc.sync.dma_start(wkv_sb[:, kc * WKK:(kc + 1) * WKK],
                                  wkvbvT.ap()[kc * P:(kc + 1) * P, :])
            nc.sync.dma_start(mask_sb[:], masks.ap()[:])
            ckv_js = []
            for j in range(NCORES):
                basek = j * CKW
                ckv_j = p2a.tile([P, 4 * TPC], BF16, tag=f"ckvj{j}",
                                 name=f"ckvj{j}")
                ckv_js.append(ckv_j)
                for r in range(4):
                    nc.sync.dma_start(ckv_j[:, r * TPC:(r + 1) * TPC],
                                      latkv_all[basek + r * P: basek + (r + 1) * P, :])
                nc.sync.dma_start(kpe2[0:DR, j * TPC:(j + 1) * TPC],
                                  latkv_all[basek + KVLR: basek + CKW, :])
                nc.sync.dma_start(kpe2[DR:P, j * TPC:(j + 1) * TPC],
                                  latkv_all[basek + KVLR: basek + CKW, :])

            # ============ Phase 1c: q up-projection for ALL heads ============
            with tc.tile_pool(name="p1qt", bufs=3) as p1qt:
                for mb in range(16):
                    ps = ps1.tile([P, TPC], F32, tag="proj", name="qup")
                    for kc in range(NQLR):
                        nc.tensor.matmul(ps[:], wqb_ch[kc][:, mb * P:(mb + 1) * P],
                                         cqn_sb[:, kc * TPC:(kc + 1) * TPC],
                                         start=(kc == 0), stop=(kc == NQLR - 1))
                    qo = p1qt.tile([P, TPC], BF16, tag="qo")
                    if mb % 2 == 0:  # pe2 block -> rope
                        _rope_dual(nc, p1qt, qo, ps, cos_sb, sin_sb, "q")
                    else:
                        nc.scalar.copy(qo[:], ps[:])
                    nc.scalar.dma_start(qa_in[mb * P:(mb + 1) * P, :], qo[:])
                nc.gpsimd.collective_compute(
                    "AllToAll", mybir.AluOpType.bypass, replica_groups=RG,
                    ins=[qa_in.opt()], outs=[qa_out.opt()])
                for mb in range(8):
                    ps = ps1.tile([P, TPC], F32, tag="proj", name="qup")
                    for kc in range(NQLR):
                        nc.tensor.matmul(ps[:], wqb_ch[kc][:, (16 + mb) * P:(17 + mb) * P],
                                         cqn_sb[:, kc * TPC:(kc + 1) * TPC],
                                         start=(kc == 0), stop=(kc == NQLR - 1))
                    qo = p1qt.tile([P, TPC], BF16, tag="qo")
                    nc.scalar.copy(qo[:], ps[:])
                    nc.scalar.dma_start(qb_in[mb * P:(mb + 1) * P, :], qo[:])
                nc.gpsimd.collective_compute(
                    "AllToAll", mybir.AluOpType.bypass, replica_groups=RG,
                    ins=[qb_in.opt()], outs=[qb_out.opt()])
            ps1_stack.close()
            p1q_stack.close()

            # ===== Phase 2: q receive + k/v up-proj (overlaps q AllToAlls) =====
            # q receives on the sync queue (idle after the phase-1 loads)
            for i in range(NCORES):
                nc.sync.dma_start(qpe[:, i * TPC:(i + 1) * TPC],
                                  qa_out[i * 2 * P: i * 2 * P + P, :])
                nc.sync.dma_start(qnope[:, i * TPC:(i + 1) * TPC],
                                  qa_out[i * 2 * P + P: (i + 1) * 2 * P, :])
            for i in range(NCORES):
                nc.sync.dma_start(qnope[:, T + i * TPC: T + (i + 1) * TPC],
                                  qb_out[i * P:(i + 1) * P, :])

            # phase-3 even-half prefetch (scalar queue; runs during kv-up)
            p3w = tc.alloc_tile_pool(name="p3w", bufs=1, side="left")
            woe_sb = p3w.tile([P, NCORES * HID], BF16)
            oe_sb = p3w.tile([P, NCORES * TPC], BF16)
            for i in range(NCORES):
                nc.scalar.dma_start(woe_sb[:, i * HID:(i + 1) * HID],
                                    woT.ap()[(2 * i) * P:(2 * i + 1) * P, :])

            with tc.tile_pool(name="ps2", bufs=4, space="PSUM") as ps2:
                # knope h0 first (unblocks even-head attention), then v for
                # both heads, then knope h1.
                for m in range(HPC):
                    for j in range(NCORES):
                        ps = ps2.tile([P, TPC], F32, tag="proj")
                        for kc in range(4):
                            nc.tensor.matmul(
                                ps[:], wkk_sb[:, kc * WKK + m * P: kc * WKK + (m + 1) * P],
                                ckv_js[j][:, kc * TPC:(kc + 1) * TPC],
                                start=(kc == 0), stop=(kc == 3))
                        nc.vector.tensor_copy(knope[:, m * T + j * TPC: m * T + (j + 1) * TPC], ps[:])
                    if m == 0:
                        for j in range(NCORES):
                            for tb in range(TPC // P):
                                ps = ps2.tile([P, WKK], F32, tag="proj")
                                for kc in range(4):
                                    nc.tensor.matmul(
                                        ps[:], ckv_js[j][:, kc * TPC + tb * P: kc * TPC + (tb + 1) * P],
                                        wkv_sb[:, kc * WKK:(kc + 1) * WKK],
                                        start=(kc == 0), stop=(kc == 3))
                                jb = j * (TPC // P) + tb
                                nc.vector.tensor_copy(v_sb[:, jb * WKK:(jb + 1) * WKK], ps[:])
            p2a.release()
            p2w.release()

            # ============ attention (4 causal units, software-pipelined) ============
            # chunk-granular: per 128-key chunk, 2 score matmuls -> exp
            # (ScalarE) -> mask/denominator (VectorE) -> v matmul. v matmuls
            # lag 2 chunks and tails lag a tile so PE-queue waits are
            # pre-satisfied and LDWEIGHTS prefetch is never blocked.
            from collections import deque
            with tc.tile_pool(name="att_t", bufs=4, side="right") as att_t, \
                 tc.tile_pool(name="att_d", bufs=2, side="right") as att_d, \
                 tc.tile_pool(name="ps_s", bufs=4, space="PSUM") as ps_s_pool, \
                 tc.tile_pool(name="ps_o", bufs=2, space="PSUM") as ps_o_pool, \
                 tc.tile_pool(name="ps_d", bufs=1, space="PSUM") as ps_d_pool:
                pend_v = deque()
                tails = deque()

                def flush_v(keep):
                    while len(pend_v) > keep:
                        pend_v.popleft()()

                for u in range(4):  # hl-major: (hl, bb)
                    hl, bb = u // 2, u % 2
                    for qt in range(QT_PER_B):
                        qoff = bb * S + qt * 512
                        ps_o = ps_o_pool.tile([P, 512], F32, tag="pso")
                        acc = att_d.tile([P, 512], F32, tag="acc")
                        nkc = 4 * (qt + 1)
                        for kc in range(nkc):
                            koff = bb * S + kc * P
                            # diagonal chunks: only queries >= mi*128 can
                            # attend these keys; narrow all work to [o0:512)
                            mi = kc - 4 * qt
                            o0 = mi * P if mi > 0 else 0
                            diag = mi >= 0
                            ps_sc = ps_s_pool.tile([P, 512], F32, tag="pss")
                            nc.tensor.matmul(
                                ps_sc[:, o0:512],
                                knope[:, hl * T + koff: hl * T + koff + P],
                                qnope[:, hl * T + qoff + o0: hl * T + qoff + 512],
                                start=True, stop=False)
                            nc.tensor.matmul(
                                ps_sc[:, o0:512],
                                kpe2[hl * DR: hl * DR + DR, koff: koff + P],
                                qpe[hl * DR: hl * DR + DR, qoff + o0: qoff + 512],
                                start=False, stop=not diag)
                            if diag:
                                # additive causal corner: tri^T @ band puts
                                # -3e4 on q-o0 < k within the corner block
                                nc.tensor.matmul(
                                    ps_sc[:, o0:o0 + P], tri_sb[:], mask_sb[:],
                                    start=False, stop=True)
                            # stagger: older v matmuls + one deferred tail
                            # step go behind this chunk's score matmuls.
                            # Tails pop only from chunk 2 on, AFTER the
                            # previous tile's last v matmuls have been
                            # emitted (ps_o must be complete before the
                            # tail's ou copy reads it).
                            flush_v(2)
                            if kc >= 2 and tails:
                                tails.popleft()()
                            ex = att_t.tile([P, 512], BF16, tag="ex")
                            nc.scalar.activation(ex[:, o0:512], ps_sc[:, o0:512],
                                                 AF.Exp)
                            if kc == 0:
                                nc.vector.tensor_copy(acc[:], ex[:])
                            else:
                                nc.vector.tensor_add(acc[:, o0:512],
                                                     acc[:, o0:512],
                                                     ex[:, o0:512])

                            def emit_v(ps_o=ps_o, ex=ex, kc=kc, bb=bb, hl=hl,
                                       nkc=nkc, o0=o0):
                                jb = bb * KB_PER_B + kc
                                nc.tensor.matmul(
                                    ps_o[:, o0:512],
                                    v_sb[:, jb * WKK + hl * DV: jb * WKK + (hl + 1) * DV],
                                    ex[:, o0:512],
                                    start=(kc == 0), stop=(kc == nkc - 1),
                                    skip_group_check=True)
                            pend_v.append(emit_v)

                        # deferred tail steps (emitted behind the next tile's
                        # first two score pairs, via the FIFO)
                        cell = {}

                        def emit_t1(acc=acc, cell=cell):
                            ps_d = ps_d_pool.tile([1, 512], F32, tag="psd", bufs=1)
                            nc.tensor.matmul(ps_d[:], ones_col_f[:], acc[:],
                                             start=True, stop=True)
                            recip = att_t.tile([1, 512], F32, tag="rcp")
                            nc.vector.reciprocal_approx_fast(recip[:], ps_d[:])
                            recip_bf = att_t.tile([1, 512], BF16, tag="rcpb")
                            nc.vector.tensor_copy(recip_bf[:], recip[:])
                            cell['r'] = recip_bf

                        def emit_t2(ps_o=ps_o, hl=hl, bb=bb, qt=qt, cell=cell):
                            recip_bf = cell['r']
                            bc = ps_d_pool.tile([P, 512], F32, tag="psd", bufs=1)
                            nc.tensor.matmul(bc[:], ones_row_bf[:], recip_bf[:],
                                             start=True, stop=True)
                            ou = att_t.tile([P, 512], F32, tag="ou", bufs=2)
                            nc.vector.tensor_copy(ou[:], ps_o[:])
                            on = att_t.tile([P, 512], BF16, tag="on")
                            nc.vector.tensor_mul(on[:], ou[:], bc[:])
                            blk = bb * QT_PER_B + qt
                            tgt = oa_in if hl == 0 else ob_in
                            nc.scalar.dma_start(tgt[blk * DV:(blk + 1) * DV, :], on[:])
                            if hl == 0 and bb == 1 and qt == QT_PER_B - 1:
                                nc.gpsimd.collective_compute(
                                    "AllToAll", mybir.AluOpType.bypass,
                                    replica_groups=RG,
                                    ins=[oa_in.opt()], outs=[oa_out.opt()])
                                # oe receives MUST be emitted after the
                                # collective so the dep tracker sees the
                                # oa_out writer (sync queue, idle here)
                                for i in range(NCORES):
                                    nc.sync.dma_start(
                                        oe_sb[:, i * TPC:(i + 1) * TPC],
                                        oa_out[i * P:(i + 1) * P, :])
                        tails.append(emit_t1)
                        tails.append(emit_t2)
                # end of stream: flush remaining
                flush_v(0)
                while tails:
                    tails.popleft()()

            att_stack.close()
            nc.gpsimd.collective_compute(
                "AllToAll", mybir.AluOpType.bypass, replica_groups=RG,
                ins=[ob_in.opt()], outs=[ob_out.opt()])

            # ============ Phase 3: two passes (pass 1 overlaps the ob AllToAll) ============
            with tc.tile_pool(name="p3t", bufs=3) as p3t, \
                 tc.tile_pool(name="ps3", bufs=4, space="PSUM") as ps3:
                part_sb = p3t.tile([P, NHID * TPC], F32, tag="part", bufs=1)
                woo_sb = p3t.tile([P, NCORES * HID], BF16, tag="woo", bufs=1)
                oo_sb = p3t.tile([P, NCORES * TPC], BF16, tag="oo", bufs=1)
                for i in range(NCORES):
                    nc.sync.dma_start(woo_sb[:, i * HID:(i + 1) * HID],
                                      woT.ap()[(2 * i + 1) * P:(2 * i + 2) * P, :])
                for i in range(NCORES):
                    nc.sync.dma_start(oo_sb[:, i * TPC:(i + 1) * TPC],
                                      ob_out[i * P:(i + 1) * P, :])
                for m in range(NHID):
                    ps = ps3.tile([P, TPC], F32, tag="proj")
                    for i in range(NCORES):
                        nc.tensor.matmul(
                            ps[:], woe_sb[:, i * HID + m * P: i * HID + (m + 1) * P],
                            oe_sb[:, i * TPC:(i + 1) * TPC],
                            start=(i == 0), stop=(i == NCORES - 1))
                    nc.scalar.copy(part_sb[:, m * TPC:(m + 1) * TPC], ps[:])
                for m in range(NHID):
                    ps = ps3.tile([P, TPC], F32, tag="proj")
                    for i in range(NCORES):
                        nc.tensor.matmul(
                            ps[:], woo_sb[:, i * HID + m * P: i * HID + (m + 1) * P],
                            oo_sb[:, i * TPC:(i + 1) * TPC],
                            start=(i == 0), stop=(i == NCORES - 1))
                    ot = p3t.tile([P, TPC], F32, tag="ot")
                    nc.vector.tensor_add(ot[:], ps[:], part_sb[:, m * TPC:(m + 1) * TPC])
                    nc.sync.dma_start(outT.ap()[m * P:(m + 1) * P, :], ot[:])
            p3w.release()
    nc.finalize()
    return nc


def _bf16(x):
    return np.ascontiguousarray(x.astype(ml_dtypes.bfloat16))


def _rope_tables():
    inv_freq = 1.0 / (THETA ** (np.arange(0, DR, 2, dtype=np.float64) / DR))
    t = np.arange(S, dtype=np.float64)
    freqs = np.outer(t, inv_freq)
    emb = np.concatenate((freqs, freqs), axis=-1)
    return np.cos(emb).astype(np.float32), np.sin(emb).astype(np.float32)


def prepare_inputs(hidden_states, w_qa, q_a_ln_w, w_qb, w_kva, kv_a_ln_w, w_kvb, w_o):
    hidden_states = np.asarray(hidden_states, dtype=np.float32)
    w_qa = np.asarray(w_qa, dtype=np.float32)
    q_a_ln_w = np.asarray(q_a_ln_w, dtype=np.float32)
    w_qb = np.asarray(w_qb, dtype=np.float32)
    w_kva = np.asarray(w_kva, dtype=np.float32)
    kv_a_ln_w = np.asarray(kv_a_ln_w, dtype=np.float32)
    w_kvb = np.asarray(w_kvb, dtype=np.float32)
    w_o = np.asarray(w_o, dtype=np.float32)

    flat = hidden_states.reshape(T, HID)
    cos, sin = _rope_tables()          # [S, DR]
    scale = DQ ** -0.5

    pos = np.arange(T) % S
    cos_d = cos[pos].T                 # [DR, T]
    sin_d = sin[pos].T

    # additive causal corner band: out[k,c] = sum_{j<=k} band[j,c] = -3e4
    # iff c < k (tri is lower-cumulative along contraction)
    BIG = -30000.0
    band = np.zeros((P, P), np.float32)
    for c in range(P - 1):
        band[c + 1, c] = BIG
    masks = _bf16(band)
    jj = np.arange(P)[:, None]
    kk = np.arange(P)[None, :]
    tri = _bf16((kk >= jj).astype(np.float32))

    w_qb_eff = (w_qb * q_a_ln_w[None, :]) * scale       # [H*DQ, QLR]
    w_kvb_eff = w_kvb * kv_a_ln_w[None, :]              # [H*(DN+DV), KVLR]

    # w_qb rows permuted: block A = per pair j [h0 pe | h1 pe | h0 nope],
    # block B = per pair j [h1 nope]
    rows = []
    for j in range(NCORES):
        h0, h1 = 2 * j, 2 * j + 1
        rows.append(w_qb_eff[h0 * DQ + DN: h0 * DQ + DQ])   # h0 pe (64)
        rows.append(w_qb_eff[h1 * DQ + DN: h1 * DQ + DQ])   # h1 pe (64)
        rows.append(w_qb_eff[h0 * DQ: h0 * DQ + DN])        # h0 nope (128)
    for j in range(NCORES):
        h1 = 2 * j + 1
        rows.append(w_qb_eff[h1 * DQ: h1 * DQ + DN])        # h1 nope (128)
    wqbT_full = _bf16(np.concatenate(rows, axis=0).T)       # [QLR, 3072]

    wqaT = _bf16(w_qa.T)
    wkvaT = _bf16(w_kva.T)
    woT = _bf16(w_o.T)

    in_maps = []
    for c in range(NCORES):
        heads = [HPC * c + h for h in range(HPC)]
        krows = [w_kvb_eff[h * (DN + DV): h * (DN + DV) + DN] for h in heads]
        wkvbkT_c = _bf16(np.concatenate(krows, axis=0).T)
        vrows = [w_kvb_eff[h * (DN + DV) + DN: (h + 1) * (DN + DV)] for h in heads]
        wkvbvT_c = _bf16(np.concatenate(vrows, axis=0).T)

        tok0 = c * TPC
        cosl = cos_d[:, tok0:tok0 + TPC]
        sinl = sin_d[:, tok0:tok0 + TPC]
        in_maps.append({
            "hidT": _bf16(flat[tok0:tok0 + TPC].T),
            "wqaT": wqaT, "wkvaT": wkvaT,
            "wqbT": wqbT_full, "wkvbkT": wkvbkT_c, "wkvbvT": wkvbvT_c,
            "woT": woT,
            "cosd": np.ascontiguousarray(np.concatenate([cosl, cosl], axis=0)),
            "sind": np.ascontiguousarray(np.concatenate([sinl, sinl], axis=0)),
            "masks": masks,
            "trid": tri,
        })
    return in_maps


def kernel(hidden_states, w_qa, q_a_ln_w, w_qb, w_kva, kv_a_ln_w, w_kvb, w_o,
           _trace=False):
    global _NC_CACHE
    if _NC_CACHE is None:
        _NC_CACHE = build_nc()
    nc = _NC_CACHE
    in_maps = prepare_inputs(hidden_states, w_qa, q_a_ln_w, w_qb, w_kva,
                             kv_a_ln_w, w_kvb, w_o)
    res = run_bass_kernel_spmd(nc, in_maps, core_ids=list(range(NCORES)),
                               trace=_trace)
    out = np.empty((T, HID), dtype=np.float32)
    for c in range(NCORES):
        out[c * TPC:(c + 1) * TPC] = res.results[c]["outT"].T
    if _trace:
        kernel._last_result = res
    return out.reshape(B, S, HID)


# revision 56
# speedup vs baseline: 1.0544x; 1.0032x over previous
"""MLA (DeepSeek-style) attention block on 8 Trainium2 NeuronCores.

Sharding:
  phase 1 (token-parallel, 8 x 512 tokens): LoRA-A down-projections + rmsnorm
    + k_pe rope; small AllGather of the kv latents (576 dims); q up-projection
    for ALL 16 heads on the token side + rope, shipped to head shards via two
    shard-aligned AllToAlls (pe+even-nope first, odd-nope second).
  phase 2 (head-parallel, 2 heads x 2 batches per core): k/v up-projection
    from gathered kv latents; causal flash attention (k-major scores, exp on
    ScalarE over chunk pairs, VectorE softmax denominator accumulation,
    software-pipelined emission so the PE never waits on exp).
  output: two AllToAlls (even heads overlap the odd-head attention; odd heads
    overlap the even half of the token-parallel output projection).

DMA queues: sync = input loads (+ phase-2/3 receives), vector = stores,
scalar = q receives + w_o prefetch. A tiny dummy AllGather at kernel start
eats the ~11us first-collective ncfw cold-start penalty.

bf16 matmuls, fp32 PSUM accumulation + softmax statistics, fp32 output.
"""
import sys
from contextlib import ExitStack

sys.path.insert(0, "/opt/trn_rl_repo")

import numpy as np
import ml_dtypes

import concourse.bacc as bacc
import concourse.mybir as mybir
import concourse.tile as tile
from concourse.bass_utils import run_bass_kernel_spmd

# ---- problem sizes (hardcoded per spec) ----
HID = 2048; H = 16; QLR = 1536; KVLR = 512
DN = 128; DR = 64; DV = 128; DQ = DN + DR
B = 2; S = 2048
THETA = 10000.0; EPS = 1e-6

NCORES = 8
T = B * S              # 4096 flattened tokens
TPC = T // NCORES      # 512 tokens per core
HPC = H // NCORES      # 2 heads per core
P = 128
NHID = HID // P        # 16
NQLR = QLR // P        # 12
CKW = KVLR + DR        # 576
QT_PER_B = S // 512    # 4 q-tiles of 512 per (b,h) unit
KB_PER_B = S // P      # 16 k-chunks of 128 per batch
WKK = HPC * DN         # 256

BF16 = mybir.dt.bfloat16
F32 = mybir.dt.float32
AF = mybir.ActivationFunctionType

_NC_CACHE = None


def _rope_dual(nc, pool, out_bf16, ps, cos_sb, sin_sb, tag):
    """RoPE on a [128, W] psum holding two 64-row head groups; writes bf16."""
    W = 512
    HDR = DR // 2
    rot = pool.tile([P, W], F32, tag=f"{tag}rot", name=f"{tag}rot", bufs=2)
    for g in range(2):
        o = g * DR
        nc.scalar.mul(rot[o:o + HDR, :], ps[o + HDR:o + DR, :], -1.0)
        nc.scalar.copy(rot[o + HDR:o + DR, :], ps[o:o + HDR, :])
    t1 = pool.tile([P, W], F32, tag=f"{tag}t1", name=f"{tag}t1", bufs=2)
    nc.vector.tensor_mul(t1[:], ps[:], cos_sb[:])
    nc.vector.tensor_mul(rot[:], rot[:], sin_sb[:])
    nc.vector.tensor_add(out_bf16[:], t1[:], rot[:])


def _phase1ab(nc, tc, ps1, ps1s, ps1b, hidT, wqaT, wkvaT, cosd, sind,
              latkv_in, latkv_all, cqn_sb, ones_col, ones_row, eps_t,
              cos_sb, sin_sb, RG, wqb_dma):
    """ckv path (+ kv AllGather) then cq path; SBUF freed on exit."""
    with tc.tile_pool(name="p1a", bufs=1) as p1a, \
         tc.tile_pool(name="p1t", bufs=2) as p1t, \
         tc.tile_pool(name="p1n", bufs=1) as p1n:
        hid_ch = [p1a.tile([P, TPC], BF16, tag=f"hid{kc}", name=f"hid{kc}")
                  for kc in range(NHID)]
        wkva_ch = [p1a.tile([P, CKW], BF16, tag=f"wkva{kc}", name=f"wkva{kc}")
                   for kc in range(NHID)]
        wqa_ch = [p1a.tile([P, QLR], BF16, tag=f"wqa{kc}", name=f"wqa{kc}")
                  for kc in range(NHID)]
        for kc in range(NHID):
            nc.sync.dma_start(hid_ch[kc][:], hidT.ap()[kc * P:(kc + 1) * P, :])
            nc.sync.dma_start(wkva_ch[kc][:], wkvaT.ap()[kc * P:(kc + 1) * P, :])
        nc.sync.dma_start(cos_sb[:], cosd.ap()[:])
        nc.sync.dma_start(sin_sb[:], sind.ap()[:])
        for kc in range(NHID):
            nc.sync.dma_start(wqa_ch[kc][:], wqaT.ap()[kc * P:(kc + 1) * P, :])
        wqb_dma()

        # --- ckv joint (kc-outer: 5 open psum groups, compute starts on
        #     the first arriving chunk) ---
        with tc.tile_pool(name="p1ckv", bufs=1) as p1ckv:
            ckv_f32 = p1ckv.tile([P, 4 * TPC], BF16)
            ssq_kv = ps1s.tile([1, TPC], F32)
            ps_m = [ps1.tile([P, TPC], F32, tag="proj", name=f"ckv{m}")
                    for m in range(4)]
            ps_pe = ps1.tile([DR, TPC], F32, tag="pe", bufs=1)
            for kc in range(NHID):
                for m in range(4):
                    nc.tensor.matmul(ps_m[m][:], wkva_ch[kc][:, m * P:(m + 1) * P],
                                     hid_ch[kc][:],
                                     start=(kc == 0), stop=(kc == NHID - 1))
                nc.tensor.matmul(ps_pe[:], wkva_ch[kc][:, KVLR:CKW],
                                 hid_ch[kc][:],
                                 start=(kc == 0), stop=(kc == NHID - 1))
            for m in range(4):
                nc.scalar.copy(ckv_f32[:, m * TPC:(m + 1) * TPC], ps_m[m][:])
                sq = p1t.tile([P, TPC], BF16, tag="sq")
                nc.vector.tensor_mul(sq[:], ckv_f32[:, m * TPC:(m + 1) * TPC],
                                     ckv_f32[:, m * TPC:(m + 1) * TPC])
                nc.tensor.matmul(ssq_kv[:], ones_col[:], sq[:],
                                 start=(m == 0), stop=(m == 3),
                                 skip_group_check=True)

            # k_pe rope (shared across heads)
            HDR = DR // 2
            rot = p1t.tile([DR, TPC], F32, tag="rot")
            nc.scalar.mul(rot[0:HDR, :], ps_pe[HDR:DR, :], -1.0)
            nc.scalar.copy(rot[HDR:DR, :], ps_pe[0:HDR, :])
            t1 = p1t.tile([DR, TPC], F32, tag="t1")
            nc.vector.tensor_mul(t1[:], ps_pe[:], cos_sb[0:DR, :])
            nc.vector.tensor_mul(rot[:], rot[:], sin_sb[0:DR, :])
            pe_out = p1t.tile([DR, TPC], BF16, tag="peo")
            nc.vector.tensor_add(pe_out[:], t1[:], rot[:])
            nc.scalar.dma_start(latkv_in[KVLR:CKW, :], pe_out[:])

            kv_norm = p1n.tile([1, TPC], F32, tag="nrm")
            nc.scalar.activation(kv_norm[:], ssq_kv[:], AF.Sqrt, bias=eps_t[:],
                                 scale=1.0 / KVLR)
            rn_kv = p1n.tile([1, TPC], F32, tag="rn")
            nc.vector.reciprocal_approx_fast(rn_kv[:], kv_norm[:])
            bkv = ps1b.tile([P, TPC], F32, tag="bc")
            nc.tensor.matmul(bkv[:], ones_row[:], rn_kv[:], start=True, stop=True)
            for m in range(4):
                lat_sb = p1t.tile([P, TPC], BF16, tag="sq")
                nc.vector.tensor_mul(lat_sb[:], ckv_f32[:, m * TPC:(m + 1) * TPC], bkv[:])
                nc.scalar.dma_start(latkv_in[m * P:(m + 1) * P, :], lat_sb[:])

            nc.gpsimd.collective_compute(
                "AllGather", mybir.AluOpType.bypass, replica_groups=RG,
                ins=[latkv_in.opt()], outs=[latkv_all.opt()])

        # --- cq (bf16 storage) + rmsnorm ---
        cq_bf = p1a.tile([P, NQLR * TPC], BF16)
        ssq_q = ps1s.tile([1, TPC], F32)
        for m in range(NQLR):
            ps = ps1.tile([P, TPC], F32, tag="proj", name="cqp")
            for kc in range(NHID):
                nc.tensor.matmul(ps[:], wqa_ch[kc][:, m * P:(m + 1) * P],
                                 hid_ch[kc][:],
                                 start=(kc == 0), stop=(kc == NHID - 1))
            nc.scalar.copy(cq_bf[:, m * TPC:(m + 1) * TPC], ps[:])
            sq = p1t.tile([P, TPC], BF16, tag="sq")
            nc.vector.tensor_mul(sq[:], cq_bf[:, m * TPC:(m + 1) * TPC],
                                 cq_bf[:, m * TPC:(m + 1) * TPC])
            nc.tensor.matmul(ssq_q[:], ones_col[:], sq[:],
                             start=(m == 0), stop=(m == NQLR - 1),
                             skip_group_check=True)
        sq_norm = p1n.tile([1, TPC], F32, tag="nrm")
        nc.scalar.activation(sq_norm[:], ssq_q[:], AF.Sqrt, bias=eps_t[:],
                             scale=1.0 / QLR)
        rn_q = p1n.tile([1, TPC], F32, tag="rn")
        nc.vector.reciprocal_approx_fast(rn_q[:], sq_norm[:])
        bq = ps1b.tile([P, TPC], F32, tag="bc")
        nc.tensor.matmul(bq[:], ones_row[:], rn_q[:], start=True, stop=True)
        for m in range(NQLR):
            nc.vector.tensor_mul(cqn_sb[:, m * TPC:(m + 1) * TPC],
                                 cq_bf[:, m * TPC:(m + 1) * TPC], bq[:])


def build_nc():
    nc = bacc.Bacc(None, target_bir_lowering=False, debug=False, num_devices=NCORES)

    # ---- per-core external inputs ----
    hidT = nc.dram_tensor("hidT", [HID, TPC], BF16, kind="ExternalInput")
    wqaT = nc.dram_tensor("wqaT", [HID, QLR], BF16, kind="ExternalInput")
    wkvaT = nc.dram_tensor("wkvaT", [HID, CKW], BF16, kind="ExternalInput")
    wqbT = nc.dram_tensor("wqbT", [QLR, H * DQ], BF16, kind="ExternalInput")
    wkvbkT = nc.dram_tensor("wkvbkT", [KVLR, HPC * DN], BF16, kind="ExternalInput")
    wkvbvT = nc.dram_tensor("wkvbvT", [KVLR, HPC * DV], BF16, kind="ExternalInput")
    woT = nc.dram_tensor("woT", [H * DV, HID], BF16, kind="ExternalInput")
    cosd = nc.dram_tensor("cosd", [P, TPC], F32, kind="ExternalInput")
    sind = nc.dram_tensor("sind", [P, TPC], F32, kind="ExternalInput")
    masks = nc.dram_tensor("masks", [P, P], BF16, kind="ExternalInput")
    trid = nc.dram_tensor("trid", [P, P], BF16, kind="ExternalInput")
    outT = nc.dram_tensor("outT", [HID, TPC], F32, kind="ExternalOutput")

    RG = [list(range(NCORES))]

    with tile.TileContext(nc) as tc:
        with tc.tile_pool(name="dram", bufs=1, space="DRAM") as dram, \
             tc.tile_pool(name="const", bufs=1) as const:
            latkv_in = dram.tile([CKW, TPC], BF16)
            latkv_all = dram.tile([NCORES * CKW, TPC], BF16, addr_space="Shared")
            qa_in = dram.tile([NCORES * 2 * P, TPC], BF16)   # [pe2|h0n] per pair
            qa_out = dram.tile([NCORES * 2 * P, TPC], BF16)
            qb_in = dram.tile([NCORES * P, TPC], BF16)       # h1n per pair
            qb_out = dram.tile([NCORES * P, TPC], BF16)
            oa_in = dram.tile([NCORES * DV, TPC], BF16)      # even heads out
            oa_out = dram.tile([NCORES * DV, TPC], BF16)
            ob_in = dram.tile([NCORES * DV, TPC], BF16)      # odd heads out
            ob_out = dram.tile([NCORES * DV, TPC], BF16)
            warm_in = dram.tile([1, 64], BF16)
            warm_out = dram.tile([NCORES, 64], BF16, addr_space="Shared")

            ones_col = const.tile([P, 1], BF16)
            nc.vector.memset(ones_col[:], 1.0)
            ones_row = const.tile([1, P], F32)
            nc.vector.memset(ones_row[:], 1.0)
            ones_row_bf = const.tile([1, P], BF16)
            nc.vector.memset(ones_row_bf[:], 1.0)
            ones_col_f = const.tile([P, 1], F32)
            nc.vector.memset(ones_col_f[:], 1.0)
            eps_t = const.tile([1, 1], F32)
            nc.vector.memset(eps_t[:], EPS)
            cos_sb = const.tile([P, TPC], F32)
            sin_sb = const.tile([P, TPC], F32)

            # warmup collective: eats the ~11us ncfw first-collective cold
            # start while phase-1a computes.
            warm_sb = const.tile([1, 64], BF16)
            nc.vector.memset(warm_sb[:], 0.0)
            nc.scalar.dma_start(warm_in[:], warm_sb[:])
            nc.gpsimd.collective_compute(
                "AllGather", mybir.AluOpType.bypass, replica_groups=RG,
                ins=[warm_in.opt()], outs=[warm_out.opt()])

            # ============ Phase 1: token-parallel compute ============
            p1q_stack = ExitStack()
            p1q = p1q_stack.enter_context(tc.tile_pool(name="p1q", bufs=1))
            ps1_stack = ExitStack()
            ps1 = ps1_stack.enter_context(tc.tile_pool(name="ps1", bufs=4, space="PSUM"))
            ps1s = ps1_stack.enter_context(tc.tile_pool(name="ps1s", bufs=1, space="PSUM"))
            ps1b = ps1_stack.enter_context(tc.tile_pool(name="ps1b", bufs=1, space="PSUM"))
            WQB = H * DQ  # 3072
            wqb_ch = [p1q.tile([P, WQB], BF16, tag=f"wqb{kc}", name=f"wqb{kc}")
                      for kc in range(NQLR)]

            def wqb_dma():
                for kc in range(NQLR):
                    nc.sync.dma_start(wqb_ch[kc][:], wqbT.ap()[kc * P:(kc + 1) * P, :])
            cqn_sb = p1q.tile([P, NQLR * TPC], BF16)
            # ---- phase 1a/1b scope (freed before q up-projection) ----
            _phase1ab(nc, tc, ps1, ps1s, ps1b, hidT, wqaT, wkvaT, cosd, sind,
                      latkv_in, latkv_all, cqn_sb, ones_col, ones_row,
                      eps_t, cos_sb, sin_sb, RG, wqb_dma)

            # ---- phase-2 receive issuance (sync queue, runs during 1c) ----
            # Long-lived attention pools go on the RIGHT side of SBUF so the
            # phase-1 pools (left) can release under them in stack order.
            att_stack = ExitStack()
            att_a = att_stack.enter_context(
                tc.tile_pool(name="att_a", bufs=1, side="right"))
            att2 = att_stack.enter_context(
                tc.tile_pool(name="att2", bufs=1, side="right"))
            p2w = tc.alloc_tile_pool(name="p2w", bufs=1, side="right")
            p2a = tc.alloc_tile_pool(name="p2a", bufs=1, side="right")
            kpe2 = att_a.tile([P, T], BF16)    # k_pe duplicated rows
            mask_sb = att_a.tile([P, P], BF16)  # additive -3e4 corner band
            tri_sb = att_a.tile([P, P], BF16)
            nc.sync.dma_start(tri_sb[:], trid.ap()[:])
            knope = att2.tile([P, 2 * T], BF16)
            v_sb = att2.tile([P, (T // P) * WKK], BF16)
            qnope = att2.tile([P, 2 * T], BF16)
            qpe = att2.tile([P, T], BF16)      # rows 0-63 h0, 64-127 h1
            wkk_sb = p2w.tile([P, 4 * WKK], BF16)
            wkv_sb = p2w.tile([P, 4 * WKK], BF16)
            for kc in range(4):
                nc.sync.dma_start(wkk_sb[:, kc * WKK:(kc + 1) * WKK],
                                  wkvbkT.ap()[kc * P:(kc + 1) * P, :])
                nc.sync.dma_start(wkv_sb[:, kc * WKK:(kc + 1) * WKK],
                                  wkvbvT.ap()[kc * P:(kc + 1) * P, :])
            nc.sync.dma_start(mask_sb[:], masks.ap()[:])
            ckv_js = []
            for j in range(NCORES):
                basek = j * CKW
                ckv_j = p2a.tile([P, 4 * TPC], BF16, tag=f"ckvj{j}",
                                 name=f"ckvj{j}")
                ckv_js.append(ckv_j)
                for r in range(4):
                    nc.sync.dma_start(ckv_j[:, r * TPC:(r + 1) * TPC],
                                      latkv_all[basek + r * P: basek + (r + 1) * P, :])
                nc.sync.dma_start(kpe2[0:DR, j * TPC:(j + 1) * TPC],
                                  latkv_all[basek + KVLR: basek + CKW, :])
                nc.sync.dma_start(kpe2[DR:P, j * TPC:(j + 1) * TPC],
                                  latkv_all[basek + KVLR: basek + CKW, :])

            # ============ Phase 1c: q up-projection for ALL heads ============
            with tc.tile_pool(name="p1qt", bufs=3) as p1qt:
                for mb in range(16):
                    ps = ps1.tile([P, TPC], F32, tag="proj", name="qup")
                    for kc in range(NQLR):
                        nc.tensor.matmul(ps[:], wqb_ch[kc][:, mb * P:(mb + 1) * P],
                                         cqn_sb[:, kc * TPC:(kc + 1) * TPC],
                                         start=(kc == 0), stop=(kc == NQLR - 1))
                    qo = p1qt.tile([P, TPC], BF16, tag="qo")
                    if mb % 2 == 0:  # pe2 block -> rope
                        _rope_dual(nc, p1qt, qo, ps, cos_sb, sin_sb, "q")
                    else:
                        nc.scalar.copy(qo[:], ps[:])
                    nc.scalar.dma_start(qa_in[mb * P:(mb + 1) * P, :], qo[:])
                nc.gpsimd.collective_compute(
                    "AllToAll", mybir.AluOpType.bypass, replica_groups=RG,
                    ins=[qa_in.opt()], outs=[qa_out.opt()])
                for mb in range(8):
                    ps = ps1.tile([P, TPC], F32, tag="proj", name="qup")
                    for kc in range(NQLR):
                        nc.tensor.matmul(ps[:], wqb_ch[kc][:, (16 + mb) * P:(17 + mb) * P],
                                         cqn_sb[:, kc * TPC:(kc + 1) * TPC],
                                         start=(kc == 0), stop=(kc == NQLR - 1))
                    qo = p1qt.tile([P, TPC], BF16, tag="qo")
                    nc.scalar.copy(qo[:], ps[:])
                    nc.scalar.dma_start(qb_in[mb * P:(mb + 1) * P, :], qo[:])
                nc.gpsimd.collective_compute(
                    "AllToAll", mybir.AluOpType.bypass, replica_groups=RG,
                    ins=[qb_in.opt()], outs=[qb_out.opt()])
            ps1_stack.close()
            p1q_stack.close()

            # ===== Phase 2: q receive + k/v up-proj (overlaps q AllToAlls) =====
            # q receives on the sync queue (idle after the phase-1 loads)
            for i in range(NCORES):
                nc.sync.dma_start(qpe[:, i * TPC:(i + 1) * TPC],
                                  qa_out[i * 2 * P: i * 2 * P + P, :])
                nc.sync.dma_start(qnope[:, i * TPC:(i + 1) * TPC],
                                  qa_out[i * 2 * P + P: (i + 1) * 2 * P, :])
            for i in range(NCORES):
                nc.sync.dma_start(qnope[:, T + i * TPC: T + (i + 1) * TPC],
                                  qb_out[i * P:(i + 1) * P, :])

            # phase-3 even-half prefetch (scalar queue; runs during kv-up)
            p3w = tc.alloc_tile_pool(name="p3w", bufs=1, side="left")
            woe_sb = p3w.tile([P, NCORES * HID], BF16)
            oe_sb = p3w.tile([P, NCORES * TPC], BF16)
            for i in range(NCORES):
                nc.scalar.dma_start(woe_sb[:, i * HID:(i + 1) * HID],
                                    woT.ap()[(2 * i) * P:(2 * i + 1) * P, :])

            with tc.tile_pool(name="ps2", bufs=4, space="PSUM") as ps2:
                # knope h0 first (unblocks even-head attention), then v for
                # both heads, then knope h1.
                for m in range(HPC):
                    for j in range(NCORES):
                        ps = ps2.tile([P, TPC], F32, tag="proj")
                        for kc in range(4):
                            nc.tensor.matmul(
                                ps[:], wkk_sb[:, kc * WKK + m * P: kc * WKK + (m + 1) * P],
                                ckv_js[j][:, kc * TPC:(kc + 1) * TPC],
                                start=(kc == 0), stop=(kc == 3))
                        nc.vector.tensor_copy(knope[:, m * T + j * TPC: m * T + (j + 1) * TPC], ps[:])
                    if m == 0:
                        for j in range(NCORES):
                            for tb in range(TPC // P):
                                ps = ps2.tile([P, WKK], F32, tag="proj")
                                for kc in range(4):
                                    nc.tensor.matmul(
                                        ps[:], ckv_js[j][:, kc * TPC + tb * P: kc * TPC + (tb + 1) * P],
                                        wkv_sb[:, kc * WKK:(kc + 1) * WKK],
                                        start=(kc == 0), stop=(kc == 3))
                                jb = j * (TPC // P) + tb
                                nc.vector.tensor_copy(v_sb[:, jb * WKK:(jb + 1) * WKK], ps[:])
            p2a.release()
            p2w.release()

            # ============ attention (4 causal units, software-pipelined) ============
            # chunk-granular: per 128-key chunk, 2 score matmuls -> exp
            # (ScalarE) -> mask/denominator (VectorE) -> v matmul. v matmuls
            # lag 2 chunks and tails lag a tile so PE-queue waits are
            # pre-satisfied and LDWEIGHTS prefetch is never blocked.
            from collections import deque
            with tc.tile_pool(name="att_t", bufs=4, side="right") as att_t, \
                 tc.tile_pool(name="att_d", bufs=2, side="right") as att_d, \
                 tc.tile_pool(name="ps_s", bufs=4, space="PSUM") as ps_s_pool, \
                 tc.tile_pool(name="ps_o", bufs=2, space="PSUM") as ps_o_pool, \
                 tc.tile_pool(name="ps_d", bufs=1, space="PSUM") as ps_d_pool:
                pend_v = deque()
                tails = deque()

                def flush_v(keep):
                    while len(pend_v) > keep:
                        pend_v.popleft()()

                for u in range(4):  # hl-major: (hl, bb)
                    hl, bb = u // 2, u % 2
                    for qt in range(QT_PER_B):
                        qoff = bb * S + qt * 512
                        ps_o = ps_o_pool.tile([P, 512], F32, tag="pso")
                        acc = att_d.tile([P, 512], F32, tag="acc")
                        nkc = 4 * (qt + 1)
                        for kc in range(nkc):
                            koff = bb * S + kc * P
                            # diagonal chunks: only queries >= mi*128 can
                            # attend these keys; narrow all work to [o0:512)
                            mi = kc - 4 * qt
                            o0 = mi * P if mi > 0 else 0
                            diag = mi >= 0
                            ps_sc = ps_s_pool.tile([P, 512], F32, tag="pss")
                            nc.tensor.matmul(
                                ps_sc[:, o0:512],
                                knope[:, hl * T + koff: hl * T + koff + P],
                                qnope[:, hl * T + qoff + o0: hl * T + qoff + 512],
                                start=True, stop=False)
                            nc.tensor.matmul(
                                ps_sc[:, o0:512],
                                kpe2[hl * DR: hl * DR + DR, koff: koff + P],
                                qpe[hl * DR: hl * DR + DR, qoff + o0: qoff + 512],
                                start=False, stop=not diag)
                            if diag:
                                # additive causal corner: tri^T @ band puts
                                # -3e4 on q-o0 < k within the corner block
                                nc.tensor.matmul(
                                    ps_sc[:, o0:o0 + P], tri_sb[:], mask_sb[:],
                                    start=False, stop=True)
                            # stagger: older v matmuls + one deferred tail
                            # step go behind this chunk's score matmuls.
                            # Tails pop only from chunk 2 on, AFTER the
                            # previous tile's last v matmuls have been
                            # emitted (ps_o must be complete before the
                            # tail's ou copy reads it).
                            flush_v(2)
                            if kc >= 2 and tails:
                                tails.popleft()()
                            ex = att_t.tile([P, 512], BF16, tag="ex", bufs=6)
                            nc.scalar.activation(ex[:, o0:512], ps_sc[:, o0:512],
                                                 AF.Exp)
                            if kc == 0:
                                nc.vector.tensor_copy(acc[:], ex[:])
                            else:
                                nc.vector.tensor_add(acc[:, o0:512],
                                                     acc[:, o0:512],
                                                     ex[:, o0:512])

                            def emit_v(ps_o=ps_o, ex=ex, kc=kc, bb=bb, hl=hl,
                                       nkc=nkc, o0=o0):
                                jb = bb * KB_PER_B + kc
                                nc.tensor.matmul(
                                    ps_o[:, o0:512],
                                    v_sb[:, jb * WKK + hl * DV: jb * WKK + (hl + 1) * DV],
                                    ex[:, o0:512],
                                    start=(kc == 0), stop=(kc == nkc - 1),
                                    skip_group_check=True)
                            pend_v.append(emit_v)

                        # deferred tail steps (emitted behind the next tile's
                        # first two score pairs, via the FIFO)
                        cell = {}

                        def emit_t1(acc=acc, cell=cell):
                            ps_d = ps_d_pool.tile([1, 512], F32, tag="psd", bufs=1)
                            nc.tensor.matmul(ps_d[:], ones_col_f[:], acc[:],
                                             start=True, stop=True)
                            recip = att_t.tile([1, 512], F32, tag="rcp")
                            nc.vector.reciprocal_approx_fast(recip[:], ps_d[:])
                            recip_bf = att_t.tile([1, 512], BF16, tag="rcpb")
                            nc.vector.tensor_copy(recip_bf[:], recip[:])
                            cell['r'] = recip_bf

                        def emit_t2(ps_o=ps_o, hl=hl, bb=bb, qt=qt, cell=cell):
                            recip_bf = cell['r']
                            bc = ps_d_pool.tile([P, 512], F32, tag="psd", bufs=1)
                            nc.tensor.matmul(bc[:], ones_row_bf[:], recip_bf[:],
                                             start=True, stop=True)
                            ou = att_t.tile([P, 512], F32, tag="ou", bufs=2)
                            nc.vector.tensor_copy(ou[:], ps_o[:])
                            on = att_t.tile([P, 512], BF16, tag="on")
                            nc.vector.tensor_mul(on[:], ou[:], bc[:])
                            blk = bb * QT_PER_B + qt
                            tgt = oa_in if hl == 0 else ob_in
                            nc.scalar.dma_start(tgt[blk * DV:(blk + 1) * DV, :], on[:])
                            if hl == 0 and bb == 1 and qt == QT_PER_B - 1:
                                nc.gpsimd.collective_compute(
                                    "AllToAll", mybir.AluOpType.bypass,
                                    replica_groups=RG,
                                    ins=[oa_in.opt()], outs=[oa_out.opt()])
                                # oe receives MUST be emitted after the
                                # collective so the dep tracker sees the
                                # oa_out writer (sync queue, idle here)
                                for i in range(NCORES):
                                    nc.sync.dma_start(
                                        oe_sb[:, i * TPC:(i + 1) * TPC],
                                        oa_out[i * P:(i + 1) * P, :])
                        tails.append(emit_t1)
                        tails.append(emit_t2)
                # end of stream: flush remaining
                flush_v(0)
                while tails:
                    tails.popleft()()

            att_stack.close()
            nc.gpsimd.collective_compute(
                "AllToAll", mybir.AluOpType.bypass, replica_groups=RG,
                ins=[ob_in.opt()], outs=[ob_out.opt()])

            # ============ Phase 3: two passes (pass 1 overlaps the ob AllToAll) ============
            with tc.tile_pool(name="p3t", bufs=3) as p3t, \
                 tc.tile_pool(name="ps3", bufs=4, space="PSUM") as ps3:
                part_sb = p3t.tile([P, NHID * TPC], F32, tag="part", bufs=1)
                woo_sb = p3t.tile([P, NCORES * HID], BF16, tag="woo", bufs=1)
                oo_sb = p3t.tile([P, NCORES * TPC], BF16, tag="oo", bufs=1)
                for i in range(NCORES):
                    nc.sync.dma_start(woo_sb[:, i * HID:(i + 1) * HID],
                                      woT.ap()[(2 * i + 1) * P:(2 * i + 2) * P, :])
                for i in range(NCORES):
                    nc.sync.dma_start(oo_sb[:, i * TPC:(i + 1) * TPC],
                                      ob_out[i * P:(i + 1) * P, :])
                for m in range(NHID):
                    ps = ps3.tile([P, TPC], F32, tag="proj")
                    for i in range(NCORES):
                        nc.tensor.matmul(
                            ps[:], woe_sb[:, i * HID + m * P: i * HID + (m + 1) * P],
                            oe_sb[:, i * TPC:(i + 1) * TPC],
                            start=(i == 0), stop=(i == NCORES - 1))
                    nc.scalar.copy(part_sb[:, m * TPC:(m + 1) * TPC], ps[:])
                for m in range(NHID):
                    ps = ps3.tile([P, TPC], F32, tag="proj")
                    for i in range(NCORES):
                        nc.tensor.matmul(
                            ps[:], woo_sb[:, i * HID + m * P: i * HID + (m + 1) * P],
                            oo_sb[:, i * TPC:(i + 1) * TPC],
                            start=(i == 0), stop=(i == NCORES - 1))
                    ot = p3t.tile([P, TPC], F32, tag="ot")
                    nc.vector.tensor_add(ot[:], ps[:], part_sb[:, m * TPC:(m + 1) * TPC])
                    nc.scalar.dma_start(outT.ap()[m * P:(m + 1) * P, :], ot[:])
            p3w.release()
    nc.finalize()
    return nc


def _bf16(x):
    return np.ascontiguousarray(x.astype(ml_dtypes.bfloat16))


def _rope_tables():
    inv_freq = 1.0 / (THETA ** (np.arange(0, DR, 2, dtype=np.float64) / DR))
    t = np.arange(S, dtype=np.float64)
    freqs = np.outer(t, inv_freq)
    emb = np.concatenate((freqs, freqs), axis=-1)
    return np.cos(emb).astype(np.float32), np.sin(emb).astype(np.float32)


def prepare_inputs(hidden_states, w_qa, q_a_ln_w, w_qb, w_kva, kv_a_ln_w, w_kvb, w_o):
    hidden_states = np.asarray(hidden_states, dtype=np.float32)
    w_qa = np.asarray(w_qa, dtype=np.float32)
    q_a_ln_w = np.asarray(q_a_ln_w, dtype=np.float32)
    w_qb = np.asarray(w_qb, dtype=np.float32)
    w_kva = np.asarray(w_kva, dtype=np.float32)
    kv_a_ln_w = np.asarray(kv_a_ln_w, dtype=np.float32)
    w_kvb = np.asarray(w_kvb, dtype=np.float32)
    w_o = np.asarray(w_o, dtype=np.float32)

    flat = hidden_states.reshape(T, HID)
    cos, sin = _rope_tables()          # [S, DR]
    scale = DQ ** -0.5

    pos = np.arange(T) % S
    cos_d = cos[pos].T                 # [DR, T]
    sin_d = sin[pos].T

    # additive causal corner band: out[k,c] = sum_{j<=k} band[j,c] = -3e4
    # iff c < k (tri is lower-cumulative along contraction)
    BIG = -30000.0
    band = np.zeros((P, P), np.float32)
    for c in range(P - 1):
        band[c + 1, c] = BIG
    masks = _bf16(band)
    jj = np.arange(P)[:, None]
    kk = np.arange(P)[None, :]
    tri = _bf16((kk >= jj).astype(np.float32))

    w_qb_eff = (w_qb * q_a_ln_w[None, :]) * scale       # [H*DQ, QLR]
    w_kvb_eff = w_kvb * kv_a_ln_w[None, :]              # [H*(DN+DV), KVLR]

    # w_qb rows permuted: block A = per pair j [h0 pe | h1 pe | h0 nope],
    # block B = per pair j [h1 nope]
    rows = []
    for j in range(NCORES):
        h0, h1 = 2 * j, 2 * j + 1
        rows.append(w_qb_eff[h0 * DQ + DN: h0 * DQ + DQ])   # h0 pe (64)
        rows.append(w_qb_eff[h1 * DQ + DN: h1 * DQ + DQ])   # h1 pe (64)
        rows.append(w_qb_eff[h0 * DQ: h0 * DQ + DN])        # h0 nope (128)
    for j in range(NCORES):
        h1 = 2 * j + 1
        rows.append(w_qb_eff[h1 * DQ: h1 * DQ + DN])        # h1 nope (128)
    wqbT_full = _bf16(np.concatenate(rows, axis=0).T)       # [QLR, 3072]

    wqaT = _bf16(w_qa.T)
    wkvaT = _bf16(w_kva.T)
    woT = _bf16(w_o.T)

    in_maps = []
    for c in range(NCORES):
        heads = [HPC * c + h for h in range(HPC)]
        krows = [w_kvb_eff[h * (DN + DV): h * (DN + DV) + DN] for h in heads]
        wkvbkT_c = _bf16(np.concatenate(krows, axis=0).T)
        vrows = [w_kvb_eff[h * (DN + DV) + DN: (h + 1) * (DN + DV)] for h in heads]
        wkvbvT_c = _bf16(np.concatenate(vrows, axis=0).T)

        tok0 = c * TPC
        cosl = cos_d[:, tok0:tok0 + TPC]
        sinl = sin_d[:, tok0:tok0 + TPC]
        in_maps.append({
            "hidT": _bf16(flat[tok0:tok0 + TPC].T),
            "wqaT": wqaT, "wkvaT": wkvaT,
            "wqbT": wqbT_full, "wkvbkT": wkvbkT_c, "wkvbvT": wkvbvT_c,
            "woT": woT,
            "cosd": np.ascontiguousarray(np.concatenate([cosl, cosl], axis=0)),
            "sind": np.ascontiguousarray(np.concatenate([sinl, sinl], axis=0)),
            "masks": masks,
            "trid": tri,
        })
    return in_maps


def kernel(hidden_states, w_qa, q_a_ln_w, w_qb, w_kva, kv_a_ln_w, w_kvb, w_o,
           _trace=False):
    global _NC_CACHE
    if _NC_CACHE is None:
        _NC_CACHE = build_nc()
    nc = _NC_CACHE
    in_maps = prepare_inputs(hidden_states, w_qa, q_a_ln_w, w_qb, w_kva,
                             kv_a_ln_w, w_kvb, w_o)
    res = run_bass_kernel_spmd(nc, in_maps, core_ids=list(range(NCORES)),
                               trace=_trace)
    out = np.empty((T, HID), dtype=np.float32)
    for c in range(NCORES):
        out[c * TPC:(c + 1) * TPC] = res.results[c]["outT"].T
    if _trace:
        kernel._last_result = res
    return out.reshape(B, S, HID)


# revision 61
# speedup vs baseline: 1.0878x; 1.0317x over previous
"""MLA (DeepSeek-style) attention block on 8 Trainium2 NeuronCores.

Sharding:
  phase 1 (token-parallel, 8 x 512 tokens): LoRA-A down-projections + rmsnorm
    + k_pe rope; small AllGather of the kv latents (576 dims); q up-projection
    for ALL 16 heads on the token side + rope, shipped to head shards via two
    shard-aligned AllToAlls (pe+even-nope first, odd-nope second).
  phase 2 (head-parallel, 2 heads x 2 batches per core): k/v up-projection
    from gathered kv latents; causal flash attention (k-major scores, exp on
    ScalarE over chunk pairs, VectorE softmax denominator accumulation,
    software-pipelined emission so the PE never waits on exp).
  output: two AllToAlls (even heads overlap the odd-head attention; odd heads
    overlap the even half of the token-parallel output projection).

DMA queues: sync = input loads (+ phase-2/3 receives), vector = stores,
scalar = q receives + w_o prefetch. A tiny dummy AllGather at kernel start
eats the ~11us first-collective ncfw cold-start penalty.

bf16 matmuls, fp32 PSUM accumulation + softmax statistics, fp32 output.
"""
import sys
from contextlib import ExitStack

sys.path.insert(0, "/opt/trn_rl_repo")

import numpy as np
import ml_dtypes

import concourse.bacc as bacc
import concourse.mybir as mybir
import concourse.tile as tile
from concourse.bass_utils import run_bass_kernel_spmd

# ---- problem sizes (hardcoded per spec) ----
HID = 2048; H = 16; QLR = 1536; KVLR = 512
DN = 128; DR = 64; DV = 128; DQ = DN + DR
B = 2; S = 2048
THETA = 10000.0; EPS = 1e-6

NCORES = 8
T = B * S              # 4096 flattened tokens
TPC = T // NCORES      # 512 tokens per core
HPC = H // NCORES      # 2 heads per core
P = 128
NHID = HID // P        # 16
NQLR = QLR // P        # 12
CKW = KVLR + DR        # 576
QT_PER_B = S // 512    # 4 q-tiles of 512 per (b,h) unit
KB_PER_B = S // P      # 16 k-chunks of 128 per batch
WKK = HPC * DN         # 256

BF16 = mybir.dt.bfloat16
F32 = mybir.dt.float32
AF = mybir.ActivationFunctionType

_NC_CACHE = None


def _rope_dual(nc, pool, out_bf16, ps, cos_sb, sin_sb, tag):
    """RoPE on a [128, W] psum holding two 64-row head groups; writes bf16."""
    W = 512
    HDR = DR // 2
    rot = pool.tile([P, W], F32, tag=f"{tag}rot", name=f"{tag}rot", bufs=2)
    for g in range(2):
        o = g * DR
        nc.scalar.mul(rot[o:o + HDR, :], ps[o + HDR:o + DR, :], -1.0)
        nc.scalar.copy(rot[o + HDR:o + DR, :], ps[o:o + HDR, :])
    t1 = pool.tile([P, W], F32, tag=f"{tag}t1", name=f"{tag}t1", bufs=2)
    nc.vector.tensor_mul(t1[:], ps[:], cos_sb[:])
    nc.vector.tensor_mul(rot[:], rot[:], sin_sb[:])
    nc.vector.tensor_add(out_bf16[:], t1[:], rot[:])


def _phase1ab(nc, tc, ps1, ps1s, hidT, wqaT, wkvaT, cosd, sind,
              latkv_in, latkv_all, cqn_sb, ones_col, ones_row, eps_t,
              cos_sb, sin_sb, RG, wqb_dma):
    """ckv path (+ kv AllGather) then cq path; SBUF freed on exit."""
    with tc.tile_pool(name="p1a", bufs=1) as p1a, \
         tc.tile_pool(name="p1t", bufs=2) as p1t, \
         tc.tile_pool(name="p1n", bufs=1) as p1n:
        hid_ch = [p1a.tile([P, TPC], BF16, tag=f"hid{kc}", name=f"hid{kc}")
                  for kc in range(NHID)]
        wkva_ch = [p1a.tile([P, CKW], BF16, tag=f"wkva{kc}", name=f"wkva{kc}")
                   for kc in range(NHID)]
        wqa_ch = [p1a.tile([P, QLR], BF16, tag=f"wqa{kc}", name=f"wqa{kc}")
                  for kc in range(NHID)]
        for kc in range(NHID):
            nc.sync.dma_start(hid_ch[kc][:], hidT.ap()[kc * P:(kc + 1) * P, :])
            nc.sync.dma_start(wkva_ch[kc][:], wkvaT.ap()[kc * P:(kc + 1) * P, :])
        nc.sync.dma_start(cos_sb[:], cosd.ap()[:])
        nc.sync.dma_start(sin_sb[:], sind.ap()[:])
        for kc in range(NHID):
            nc.sync.dma_start(wqa_ch[kc][:], wqaT.ap()[kc * P:(kc + 1) * P, :])
        wqb_dma()

        # --- ckv joint (kc-outer: 5 open psum groups, compute starts on
        #     the first arriving chunk) ---
        with tc.tile_pool(name="p1ckv", bufs=1) as p1ckv:
            ckv_f32 = p1ckv.tile([P, 4 * TPC], BF16)
            ssq_kv = ps1s.tile([1, TPC], F32)
            ps_m = [ps1.tile([P, TPC], F32, tag="proj", name=f"ckv{m}")
                    for m in range(4)]
            ps_pe = ps1.tile([DR, TPC], F32, tag="pebc", bufs=1)
            for kc in range(NHID):
                for m in range(4):
                    nc.tensor.matmul(ps_m[m][:], wkva_ch[kc][:, m * P:(m + 1) * P],
                                     hid_ch[kc][:],
                                     start=(kc == 0), stop=(kc == NHID - 1))
                nc.tensor.matmul(ps_pe[:], wkva_ch[kc][:, KVLR:CKW],
                                 hid_ch[kc][:],
                                 start=(kc == 0), stop=(kc == NHID - 1))
            for m in range(4):
                nc.scalar.copy(ckv_f32[:, m * TPC:(m + 1) * TPC], ps_m[m][:])
                sq = p1t.tile([P, TPC], BF16, tag="sq")
                nc.vector.tensor_mul(sq[:], ckv_f32[:, m * TPC:(m + 1) * TPC],
                                     ckv_f32[:, m * TPC:(m + 1) * TPC])
                nc.tensor.matmul(ssq_kv[:], ones_col[:], sq[:],
                                 start=(m == 0), stop=(m == 3),
                                 skip_group_check=True)

            # k_pe rope (shared across heads)
            HDR = DR // 2
            rot = p1t.tile([DR, TPC], F32, tag="rot")
            nc.scalar.mul(rot[0:HDR, :], ps_pe[HDR:DR, :], -1.0)
            nc.scalar.copy(rot[HDR:DR, :], ps_pe[0:HDR, :])
            t1 = p1t.tile([DR, TPC], F32, tag="t1")
            nc.vector.tensor_mul(t1[:], ps_pe[:], cos_sb[0:DR, :])
            nc.vector.tensor_mul(rot[:], rot[:], sin_sb[0:DR, :])
            pe_out = p1t.tile([DR, TPC], BF16, tag="peo")
            nc.vector.tensor_add(pe_out[:], t1[:], rot[:])
            nc.scalar.dma_start(latkv_in[KVLR:CKW, :], pe_out[:])

            kv_norm = p1n.tile([1, TPC], F32, tag="nrm")
            nc.scalar.activation(kv_norm[:], ssq_kv[:], AF.Sqrt, bias=eps_t[:],
                                 scale=1.0 / KVLR)
            rn_kv = p1n.tile([1, TPC], F32, tag="rn")
            nc.vector.reciprocal_approx_fast(rn_kv[:], kv_norm[:])
            bkv = ps1.tile([P, TPC], F32, tag="pebc", bufs=1, name="bkv")
            nc.tensor.matmul(bkv[:], ones_row[:], rn_kv[:], start=True, stop=True)
            for m in range(4):
                lat_sb = p1t.tile([P, TPC], BF16, tag="sq")
                nc.vector.tensor_mul(lat_sb[:], ckv_f32[:, m * TPC:(m + 1) * TPC], bkv[:])
                nc.scalar.dma_start(latkv_in[m * P:(m + 1) * P, :], lat_sb[:])

            nc.gpsimd.collective_compute(
                "AllGather", mybir.AluOpType.bypass, replica_groups=RG,
                ins=[latkv_in.opt()], outs=[latkv_all.opt()])

        # --- cq (bf16 storage) + rmsnorm ---
        cq_bf = p1a.tile([P, NQLR * TPC], BF16)
        ssq_q = ps1s.tile([1, TPC], F32)
        for m in range(NQLR):
            ps = ps1.tile([P, TPC], F32, tag="proj", name="cqp")
            for kc in range(NHID):
                nc.tensor.matmul(ps[:], wqa_ch[kc][:, m * P:(m + 1) * P],
                                 hid_ch[kc][:],
                                 start=(kc == 0), stop=(kc == NHID - 1))
            nc.scalar.copy(cq_bf[:, m * TPC:(m + 1) * TPC], ps[:])
            sq = p1t.tile([P, TPC], BF16, tag="sq")
            nc.vector.tensor_mul(sq[:], cq_bf[:, m * TPC:(m + 1) * TPC],
                                 cq_bf[:, m * TPC:(m + 1) * TPC])
            nc.tensor.matmul(ssq_q[:], ones_col[:], sq[:],
                             start=(m == 0), stop=(m == NQLR - 1),
                             skip_group_check=True)
        sq_norm = p1n.tile([1, TPC], F32, tag="nrm")
        nc.scalar.activation(sq_norm[:], ssq_q[:], AF.Sqrt, bias=eps_t[:],
                             scale=1.0 / QLR)
        rn_q = p1n.tile([1, TPC], F32, tag="rn")
        nc.vector.reciprocal_approx_fast(rn_q[:], sq_norm[:])
        bq = ps1.tile([P, TPC], F32, tag="pebc", bufs=1, name="bq")
        nc.tensor.matmul(bq[:], ones_row[:], rn_q[:], start=True, stop=True)
        for m in range(NQLR):
            nc.vector.tensor_mul(cqn_sb[:, m * TPC:(m + 1) * TPC],
                                 cq_bf[:, m * TPC:(m + 1) * TPC], bq[:])


def build_nc():
    nc = bacc.Bacc(None, target_bir_lowering=False, debug=False, num_devices=NCORES)

    # ---- per-core external inputs ----
    hidT = nc.dram_tensor("hidT", [HID, TPC], BF16, kind="ExternalInput")
    wqaT = nc.dram_tensor("wqaT", [HID, QLR], BF16, kind="ExternalInput")
    wkvaT = nc.dram_tensor("wkvaT", [HID, CKW], BF16, kind="ExternalInput")
    wqbT = nc.dram_tensor("wqbT", [QLR, H * DQ], BF16, kind="ExternalInput")
    wkvbkT = nc.dram_tensor("wkvbkT", [KVLR, HPC * DN], BF16, kind="ExternalInput")
    wkvbvT = nc.dram_tensor("wkvbvT", [KVLR, HPC * DV], BF16, kind="ExternalInput")
    woT = nc.dram_tensor("woT", [H * DV, HID], BF16, kind="ExternalInput")
    cosd = nc.dram_tensor("cosd", [P, TPC], F32, kind="ExternalInput")
    sind = nc.dram_tensor("sind", [P, TPC], F32, kind="ExternalInput")
    masks = nc.dram_tensor("masks", [P, P], BF16, kind="ExternalInput")
    trid = nc.dram_tensor("trid", [P, P], BF16, kind="ExternalInput")
    outT = nc.dram_tensor("outT", [HID, TPC], F32, kind="ExternalOutput")

    RG = [list(range(NCORES))]

    with tile.TileContext(nc) as tc:
        with tc.tile_pool(name="dram", bufs=1, space="DRAM") as dram, \
             tc.tile_pool(name="const", bufs=1) as const:
            latkv_in = dram.tile([CKW, TPC], BF16)
            latkv_all = dram.tile([NCORES * CKW, TPC], BF16, addr_space="Shared")
            qa_in = dram.tile([NCORES * 2 * P, TPC], BF16)   # [pe2|h0n] per pair
            qa_out = dram.tile([NCORES * 2 * P, TPC], BF16)
            qb_in = dram.tile([NCORES * P, TPC], BF16)       # h1n per pair
            qb_out = dram.tile([NCORES * P, TPC], BF16)
            oa_in = dram.tile([NCORES * DV, TPC], BF16)      # even heads out
            oa_out = dram.tile([NCORES * DV, TPC], BF16)
            ob_in = dram.tile([NCORES * DV, TPC], BF16)      # odd heads out
            ob_out = dram.tile([NCORES * DV, TPC], BF16)
            warm_in = dram.tile([1, 64], BF16)
            warm_out = dram.tile([NCORES, 64], BF16, addr_space="Shared")

            ones_col = const.tile([P, 1], BF16)
            nc.vector.memset(ones_col[:], 1.0)
            ones_row = const.tile([1, P], F32)
            nc.vector.memset(ones_row[:], 1.0)
            ones_row_bf = const.tile([1, P], BF16)
            nc.vector.memset(ones_row_bf[:], 1.0)
            ones_col_f = const.tile([P, 1], F32)
            nc.vector.memset(ones_col_f[:], 1.0)
            eps_t = const.tile([1, 1], F32)
            nc.vector.memset(eps_t[:], EPS)
            cos_sb = const.tile([P, TPC], F32)
            sin_sb = const.tile([P, TPC], F32)

            # warmup collective: eats the ~11us ncfw first-collective cold
            # start while phase-1a computes.
            warm_sb = const.tile([1, 64], BF16)
            nc.vector.memset(warm_sb[:], 0.0)
            nc.scalar.dma_start(warm_in[:], warm_sb[:])
            nc.gpsimd.collective_compute(
                "AllGather", mybir.AluOpType.bypass, replica_groups=RG,
                ins=[warm_in.opt()], outs=[warm_out.opt()])

            # ============ Phase 1: token-parallel compute ============
            p1q_stack = ExitStack()
            p1q = p1q_stack.enter_context(tc.tile_pool(name="p1q", bufs=1))
            ps1_stack = ExitStack()
            ps1 = ps1_stack.enter_context(tc.tile_pool(name="ps1", bufs=5, space="PSUM"))
            ps1s = ps1_stack.enter_context(tc.tile_pool(name="ps1s", bufs=1, space="PSUM"))
            WQB = H * DQ  # 3072
            wqb_ch = [p1q.tile([P, WQB], BF16, tag=f"wqb{kc}", name=f"wqb{kc}")
                      for kc in range(NQLR)]

            def wqb_dma():
                for kc in range(NQLR):
                    nc.sync.dma_start(wqb_ch[kc][:], wqbT.ap()[kc * P:(kc + 1) * P, :])
            cqn_sb = p1q.tile([P, NQLR * TPC], BF16)
            # ---- phase 1a/1b scope (freed before q up-projection) ----
            _phase1ab(nc, tc, ps1, ps1s, hidT, wqaT, wkvaT, cosd, sind,
                      latkv_in, latkv_all, cqn_sb, ones_col, ones_row,
                      eps_t, cos_sb, sin_sb, RG, wqb_dma)

            # ---- phase-2 receive issuance (sync queue, runs during 1c) ----
            # Long-lived attention pools go on the RIGHT side of SBUF so the
            # phase-1 pools (left) can release under them in stack order.
            att_stack = ExitStack()
            att_a = att_stack.enter_context(
                tc.tile_pool(name="att_a", bufs=1, side="right"))
            att2 = att_stack.enter_context(
                tc.tile_pool(name="att2", bufs=1, side="right"))
            p2w = tc.alloc_tile_pool(name="p2w", bufs=1, side="right")
            p2a = tc.alloc_tile_pool(name="p2a", bufs=1, side="right")
            kpe2 = att_a.tile([P, T], BF16)    # k_pe duplicated rows
            mask_sb = att_a.tile([P, P], BF16)  # additive -3e4 corner band
            tri_sb = att_a.tile([P, P], BF16)
            nc.sync.dma_start(tri_sb[:], trid.ap()[:])
            knope = att2.tile([P, 2 * T], BF16)
            v_sb = att2.tile([P, (T // P) * WKK], BF16)
            qnope = att2.tile([P, 2 * T], BF16)
            qpe = att2.tile([P, T], BF16)      # rows 0-63 h0, 64-127 h1
            wkk_sb = p2w.tile([P, 4 * WKK], BF16)
            wkv_sb = p2w.tile([P, 4 * WKK], BF16)
            for kc in range(4):
                nc.sync.dma_start(wkk_sb[:, kc * WKK:(kc + 1) * WKK],
                                  wkvbkT.ap()[kc * P:(kc + 1) * P, :])
                nc.sync.dma_start(wkv_sb[:, kc * WKK:(kc + 1) * WKK],
                                  wkvbvT.ap()[kc * P:(kc + 1) * P, :])
            nc.sync.dma_start(mask_sb[:], masks.ap()[:])
            ckv_js = []
            for j in range(NCORES):
                basek = j * CKW
                ckv_j = p2a.tile([P, 4 * TPC], BF16, tag=f"ckvj{j}",
                                 name=f"ckvj{j}")
                ckv_js.append(ckv_j)
                for r in range(4):
                    nc.sync.dma_start(ckv_j[:, r * TPC:(r + 1) * TPC],
                                      latkv_all[basek + r * P: basek + (r + 1) * P, :])
                nc.sync.dma_start(kpe2[0:DR, j * TPC:(j + 1) * TPC],
                                  latkv_all[basek + KVLR: basek + CKW, :])
                nc.sync.dma_start(kpe2[DR:P, j * TPC:(j + 1) * TPC],
                                  latkv_all[basek + KVLR: basek + CKW, :])

            # ============ Phase 1c: q up-projection for ALL heads ============
            with tc.tile_pool(name="p1qt", bufs=3) as p1qt:
                for mb in range(16):
                    ps = ps1.tile([P, TPC], F32, tag="proj", name="qup")
                    for kc in range(NQLR):
                        nc.tensor.matmul(ps[:], wqb_ch[kc][:, mb * P:(mb + 1) * P],
                                         cqn_sb[:, kc * TPC:(kc + 1) * TPC],
                                         start=(kc == 0), stop=(kc == NQLR - 1))
                    qo = p1qt.tile([P, TPC], BF16, tag="qo")
                    if mb % 2 == 0:  # pe2 block -> rope
                        _rope_dual(nc, p1qt, qo, ps, cos_sb, sin_sb, "q")
                    else:
                        nc.scalar.copy(qo[:], ps[:])
                    nc.scalar.dma_start(qa_in[mb * P:(mb + 1) * P, :], qo[:])
                nc.gpsimd.collective_compute(
                    "AllToAll", mybir.AluOpType.bypass, replica_groups=RG,
                    ins=[qa_in.opt()], outs=[qa_out.opt()])
                for mb in range(8):
                    ps = ps1.tile([P, TPC], F32, tag="proj", name="qup")
                    for kc in range(NQLR):
                        nc.tensor.matmul(ps[:], wqb_ch[kc][:, (16 + mb) * P:(17 + mb) * P],
                                         cqn_sb[:, kc * TPC:(kc + 1) * TPC],
                                         start=(kc == 0), stop=(kc == NQLR - 1))
                    qo = p1qt.tile([P, TPC], BF16, tag="qo")
                    nc.scalar.copy(qo[:], ps[:])
                    nc.scalar.dma_start(qb_in[mb * P:(mb + 1) * P, :], qo[:])
                nc.gpsimd.collective_compute(
                    "AllToAll", mybir.AluOpType.bypass, replica_groups=RG,
                    ins=[qb_in.opt()], outs=[qb_out.opt()])
            ps1_stack.close()
            p1q_stack.close()

            # ===== Phase 2: q receive + k/v up-proj (overlaps q AllToAlls) =====
            # q receives on the sync queue (idle after the phase-1 loads)
            for i in range(NCORES):
                nc.sync.dma_start(qpe[:, i * TPC:(i + 1) * TPC],
                                  qa_out[i * 2 * P: i * 2 * P + P, :])
                nc.sync.dma_start(qnope[:, i * TPC:(i + 1) * TPC],
                                  qa_out[i * 2 * P + P: (i + 1) * 2 * P, :])
            for i in range(NCORES):
                nc.sync.dma_start(qnope[:, T + i * TPC: T + (i + 1) * TPC],
                                  qb_out[i * P:(i + 1) * P, :])

            # phase-3 even-half prefetch (scalar queue; runs during kv-up)
            p3w = tc.alloc_tile_pool(name="p3w", bufs=1, side="left")
            woe_sb = p3w.tile([P, NCORES * HID], BF16)
            oe_sb = p3w.tile([P, NCORES * TPC], BF16)
            for i in range(NCORES):
                nc.scalar.dma_start(woe_sb[:, i * HID:(i + 1) * HID],
                                    woT.ap()[(2 * i) * P:(2 * i + 1) * P, :])

            with tc.tile_pool(name="ps2", bufs=4, space="PSUM") as ps2:
                # knope h0 first (unblocks even-head attention), then v for
                # both heads, then knope h1.
                for m in range(HPC):
                    for j in range(NCORES):
                        ps = ps2.tile([P, TPC], F32, tag="proj")
                        for kc in range(4):
                            nc.tensor.matmul(
                                ps[:], wkk_sb[:, kc * WKK + m * P: kc * WKK + (m + 1) * P],
                                ckv_js[j][:, kc * TPC:(kc + 1) * TPC],
                                start=(kc == 0), stop=(kc == 3))
                        nc.vector.tensor_copy(knope[:, m * T + j * TPC: m * T + (j + 1) * TPC], ps[:])
                    if m == 0:
                        for j in range(NCORES):
                            for tb in range(TPC // P):
                                ps = ps2.tile([P, WKK], F32, tag="proj")
                                for kc in range(4):
                                    nc.tensor.matmul(
                                        ps[:], ckv_js[j][:, kc * TPC + tb * P: kc * TPC + (tb + 1) * P],
                                        wkv_sb[:, kc * WKK:(kc + 1) * WKK],
                                        start=(kc == 0), stop=(kc == 3))
                                jb = j * (TPC // P) + tb
                                nc.vector.tensor_copy(v_sb[:, jb * WKK:(jb + 1) * WKK], ps[:])
            p2a.release()
            p2w.release()

            # ============ attention (4 causal units, software-pipelined) ============
            # chunk-granular: per 128-key chunk, 2 score matmuls -> exp
            # (ScalarE) -> mask/denominator (VectorE) -> v matmul. v matmuls
            # lag 2 chunks and tails lag a tile so PE-queue waits are
            # pre-satisfied and LDWEIGHTS prefetch is never blocked.
            from collections import deque
            with tc.tile_pool(name="att_t", bufs=4, side="right") as att_t, \
                 tc.tile_pool(name="att_d", bufs=2, side="right") as att_d, \
                 tc.tile_pool(name="ps_s", bufs=4, space="PSUM") as ps_s_pool, \
                 tc.tile_pool(name="ps_o", bufs=2, space="PSUM") as ps_o_pool, \
                 tc.tile_pool(name="ps_d", bufs=1, space="PSUM") as ps_d_pool:
                pend_v = deque()
                tails = deque()

                def flush_v(keep):
                    while len(pend_v) > keep:
                        pend_v.popleft()()

                for u in range(4):  # hl-major: (hl, bb)
                    hl, bb = u // 2, u % 2
                    for qt in range(QT_PER_B):
                        qoff = bb * S + qt * 512
                        ps_o = ps_o_pool.tile([P, 512], F32, tag="pso")
                        acc = att_d.tile([P, 512], F32, tag="acc")
                        nkc = 4 * (qt + 1)
                        for kc in range(nkc):
                            koff = bb * S + kc * P
                            # diagonal chunks: only queries >= mi*128 can
                            # attend these keys; narrow all work to [o0:512)
                            mi = kc - 4 * qt
                            o0 = mi * P if mi > 0 else 0
                            diag = mi >= 0
                            ps_sc = ps_s_pool.tile([P, 512], F32, tag="pss")
                            nc.tensor.matmul(
                                ps_sc[:, o0:512],
                                knope[:, hl * T + koff: hl * T + koff + P],
                                qnope[:, hl * T + qoff + o0: hl * T + qoff + 512],
                                start=True, stop=False)
                            nc.tensor.matmul(
                                ps_sc[:, o0:512],
                                kpe2[hl * DR: hl * DR + DR, koff: koff + P],
                                qpe[hl * DR: hl * DR + DR, qoff + o0: qoff + 512],
                                start=False, stop=not diag)
                            if diag:
                                # additive causal corner: tri^T @ band puts
                                # -3e4 on q-o0 < k within the corner block
                                nc.tensor.matmul(
                                    ps_sc[:, o0:o0 + P], tri_sb[:], mask_sb[:],
                                    start=False, stop=True)
                            # stagger: older v matmuls + one deferred tail
                            # step go behind this chunk's score matmuls.
                            # Tails pop only from chunk 2 on, AFTER the
                            # previous tile's last v matmuls have been
                            # emitted (ps_o must be complete before the
                            # tail's ou copy reads it).
                            flush_v(2)
                            if kc >= 2 and tails:
                                tails.popleft()()
                            ex = att_t.tile([P, 512], BF16, tag="ex", bufs=6)
                            nc.scalar.activation(ex[:, o0:512], ps_sc[:, o0:512],
                                                 AF.Exp)
                            if kc == 0:
                                nc.vector.tensor_copy(acc[:], ex[:])
                            else:
                                nc.vector.tensor_add(acc[:, o0:512],
                                                     acc[:, o0:512],
                                                     ex[:, o0:512])

                            def emit_v(ps_o=ps_o, ex=ex, kc=kc, bb=bb, hl=hl,
                                       nkc=nkc, o0=o0):
                                jb = bb * KB_PER_B + kc
                                nc.tensor.matmul(
                                    ps_o[:, o0:512],
                                    v_sb[:, jb * WKK + hl * DV: jb * WKK + (hl + 1) * DV],
                                    ex[:, o0:512],
                                    start=(kc == 0), stop=(kc == nkc - 1),
                                    skip_group_check=True)
                            pend_v.append(emit_v)

                        # deferred tail steps (emitted behind the next tile's
                        # first two score pairs, via the FIFO)
                        cell = {}

                        def emit_t1(acc=acc, cell=cell):
                            ps_d = ps_d_pool.tile([1, 512], F32, tag="psd", bufs=1)
                            nc.tensor.matmul(ps_d[:], ones_col_f[:], acc[:],
                                             start=True, stop=True)
                            recip = att_t.tile([1, 512], F32, tag="rcp")
                            nc.vector.reciprocal_approx_fast(recip[:], ps_d[:])
                            recip_bf = att_t.tile([1, 512], BF16, tag="rcpb")
                            nc.vector.tensor_copy(recip_bf[:], recip[:])
                            cell['r'] = recip_bf

                        def emit_t2(ps_o=ps_o, hl=hl, bb=bb, qt=qt, cell=cell):
                            recip_bf = cell['r']
                            bc = ps_d_pool.tile([P, 512], F32, tag="psd", bufs=1)
                            nc.tensor.matmul(bc[:], ones_row_bf[:], recip_bf[:],
                                             start=True, stop=True)
                            ou = att_t.tile([P, 512], F32, tag="ou", bufs=2)
                            nc.vector.tensor_copy(ou[:], ps_o[:])
                            on = att_t.tile([P, 512], BF16, tag="on")
                            nc.vector.tensor_mul(on[:], ou[:], bc[:])
                            blk = bb * QT_PER_B + qt
                            tgt = oa_in if hl == 0 else ob_in
                            nc.scalar.dma_start(tgt[blk * DV:(blk + 1) * DV, :], on[:])
                            if hl == 0 and bb == 1 and qt == QT_PER_B - 1:
                                nc.gpsimd.collective_compute(
                                    "AllToAll", mybir.AluOpType.bypass,
                                    replica_groups=RG,
                                    ins=[oa_in.opt()], outs=[oa_out.opt()])
                                # oe receives MUST be emitted after the
                                # collective so the dep tracker sees the
                                # oa_out writer (sync queue, idle here)
                                for i in range(NCORES):
                                    nc.sync.dma_start(
                                        oe_sb[:, i * TPC:(i + 1) * TPC],
                                        oa_out[i * P:(i + 1) * P, :])
                        tails.append(emit_t1)
                        tails.append(emit_t2)
                # end of stream: flush remaining
                flush_v(0)
                while tails:
                    tails.popleft()()

            att_stack.close()
            nc.gpsimd.collective_compute(
                "AllToAll", mybir.AluOpType.bypass, replica_groups=RG,
                ins=[ob_in.opt()], outs=[ob_out.opt()])

            # ============ Phase 3: two passes (pass 1 overlaps the ob AllToAll) ============
            with tc.tile_pool(name="p3t", bufs=3) as p3t, \
                 tc.tile_pool(name="ps3", bufs=4, space="PSUM") as ps3:
                part_sb = p3t.tile([P, NHID * TPC], F32, tag="part", bufs=1)
                woo_sb = p3t.tile([P, NCORES * HID], BF16, tag="woo", bufs=1)
                oo_sb = p3t.tile([P, NCORES * TPC], BF16, tag="oo", bufs=1)
                for i in range(NCORES):
                    nc.sync.dma_start(woo_sb[:, i * HID:(i + 1) * HID],
                                      woT.ap()[(2 * i + 1) * P:(2 * i + 2) * P, :])
                for i in range(NCORES):
                    nc.sync.dma_start(oo_sb[:, i * TPC:(i + 1) * TPC],
                                      ob_out[i * P:(i + 1) * P, :])
                for m in range(NHID):
                    ps = ps3.tile([P, TPC], F32, tag="proj")
                    for i in range(NCORES):
                        nc.tensor.matmul(
                            ps[:], woe_sb[:, i * HID + m * P: i * HID + (m + 1) * P],
                            oe_sb[:, i * TPC:(i + 1) * TPC],
                            start=(i == 0), stop=(i == NCORES - 1))
                    nc.scalar.copy(part_sb[:, m * TPC:(m + 1) * TPC], ps[:])
                for m in range(NHID):
                    ps = ps3.tile([P, TPC], F32, tag="proj")
                    for i in range(NCORES):
                        nc.tensor.matmul(
                            ps[:], woo_sb[:, i * HID + m * P: i * HID + (m + 1) * P],
                            oo_sb[:, i * TPC:(i + 1) * TPC],
                            start=(i == 0), stop=(i == NCORES - 1))
                    ot = p3t.tile([P, TPC], F32, tag="ot")
                    nc.vector.tensor_add(ot[:], ps[:], part_sb[:, m * TPC:(m + 1) * TPC])
                    nc.scalar.dma_start(outT.ap()[m * P:(m + 1) * P, :], ot[:])
            p3w.release()
    nc.finalize()
    return nc


def _bf16(x):
    return np.ascontiguousarray(x.astype(ml_dtypes.bfloat16))


def _rope_tables():
    inv_freq = 1.0 / (THETA ** (np.arange(0, DR, 2, dtype=np.float64) / DR))
    t = np.arange(S, dtype=np.float64)
    freqs = np.outer(t, inv_freq)
    emb = np.concatenate((freqs, freqs), axis=-1)
    return np.cos(emb).astype(np.float32), np.sin(emb).astype(np.float32)


def prepare_inputs(hidden_states, w_qa, q_a_ln_w, w_qb, w_kva, kv_a_ln_w, w_kvb, w_o):
    hidden_states = np.asarray(hidden_states, dtype=np.float32)
    w_qa = np.asarray(w_qa, dtype=np.float32)
    q_a_ln_w = np.asarray(q_a_ln_w, dtype=np.float32)
    w_qb = np.asarray(w_qb, dtype=np.float32)
    w_kva = np.asarray(w_kva, dtype=np.float32)
    kv_a_ln_w = np.asarray(kv_a_ln_w, dtype=np.float32)
    w_kvb = np.asarray(w_kvb, dtype=np.float32)
    w_o = np.asarray(w_o, dtype=np.float32)

    flat = hidden_states.reshape(T, HID)
    cos, sin = _rope_tables()          # [S, DR]
    scale = DQ ** -0.5

    pos = np.arange(T) % S
    cos_d = cos[pos].T                 # [DR, T]
    sin_d = sin[pos].T

    # additive causal corner band: out[k,c] = sum_{j<=k} band[j,c] = -3e4
    # iff c < k (tri is lower-cumulative along contraction)
    BIG = -30000.0
    band = np.zeros((P, P), np.float32)
    for c in range(P - 1):
        band[c + 1, c] = BIG
    masks = _bf16(band)
    jj = np.arange(P)[:, None]
    kk = np.arange(P)[None, :]
    tri = _bf16((kk >= jj).astype(np.float32))

    w_qb_eff = (w_qb * q_a_ln_w[None, :]) * scale       # [H*DQ, QLR]
    w_kvb_eff = w_kvb * kv_a_ln_w[None, :]              # [H*(DN+DV), KVLR]

    # w_qb rows permuted: block A = per pair j [h0 pe | h1 pe | h0 nope],
    # block B = per pair j [h1 nope]
    rows = []
    for j in range(NCORES):
        h0, h1 = 2 * j, 2 * j + 1
        rows.append(w_qb_eff[h0 * DQ + DN: h0 * DQ + DQ])   # h0 pe (64)
        rows.append(w_qb_eff[h1 * DQ + DN: h1 * DQ + DQ])   # h1 pe (64)
        rows.append(w_qb_eff[h0 * DQ: h0 * DQ + DN])        # h0 nope (128)
    for j in range(NCORES):
        h1 = 2 * j + 1
        rows.append(w_qb_eff[h1 * DQ: h1 * DQ + DN])        # h1 nope (128)
    wqbT_full = _bf16(np.concatenate(rows, axis=0).T)       # [QLR, 3072]

    wqaT = _bf16(w_qa.T)
    wkvaT = _bf16(w_kva.T)
    woT = _bf16(w_o.T)

    in_maps = []
    for c in range(NCORES):
        heads = [HPC * c + h for h in range(HPC)]
        krows = [w_kvb_eff[h * (DN + DV): h * (DN + DV) + DN] for h in heads]
        wkvbkT_c = _bf16(np.concatenate(krows, axis=0).T)
        vrows = [w_kvb_eff[h * (DN + DV) + DN: (h + 1) * (DN + DV)] for h in heads]
        wkvbvT_c = _bf16(np.concatenate(vrows, axis=0).T)

        tok0 = c * TPC
        cosl = cos_d[:, tok0:tok0 + TPC]
        sinl = sin_d[:, tok0:tok0 + TPC]
        in_maps.append({
            "hidT": _bf16(flat[tok0:tok0 + TPC].T),
            "wqaT": wqaT, "wkvaT": wkvaT,
            "wqbT": wqbT_full, "wkvbkT": wkvbkT_c, "wkvbvT": wkvbvT_c,
            "woT": woT,
            "cosd": np.ascontiguousarray(np.concatenate([cosl, cosl], axis=0)),
            "sind": np.ascontiguousarray(np.concatenate([sinl, sinl], axis=0)),
            "masks": masks,
            "trid": tri,
        })
    return in_maps


def kernel(hidden_states, w_qa, q_a_ln_w, w_qb, w_kva, kv_a_ln_w, w_kvb, w_o,
           _trace=False):
    global _NC_CACHE
    if _NC_CACHE is None:
        _NC_CACHE = build_nc()
    nc = _NC_CACHE
    in_maps = prepare_inputs(hidden_states, w_qa, q_a_ln_w, w_qb, w_kva,
                             kv_a_ln_w, w_kvb, w_o)
    res = run_bass_kernel_spmd(nc, in_maps, core_ids=list(range(NCORES)),
                               trace=_trace)
    out = np.empty((T, HID), dtype=np.float32)
    for c in range(NCORES):
        out[c * TPC:(c + 1) * TPC] = res.results[c]["outT"].T
    if _trace:
        kernel._last_result = res
    return out.reshape(B, S, HID)
